# revision 35
# baseline (speedup 1.0000x reference)
"""Trainium2 Bass kernel for the LSTM seq2seq autoencoder (layout B).

Strategy:
  - Data-parallel over batch: B=512 -> 64 rows per core on 8 cores.
  - Gates-on-partitions layout: gate preactivations live in one PSUM bank
    [128, 512] = 8 chunks x 64 batch cols, chunk order [i0 i1 f0 f1 o0 o1 g0 g1].
    Each step: 16 (h) + 8 (x, encoder) LDW+MM pairs of N=64 (FWL-eligible
    bf16 weights, LDWEIGHTS hidden behind matmuls) + one rank-8 bias matmul
    (decoder) whose rhs is a block-indicator [8, 512].
  - h^T [128, 2, 64] is produced directly by the DVE h-mul (no per-step
    transposes or PSUM->SBUF copies) and is the rhs of the next step's MMs.
  - Encoder length masking: c frozen by forcing i -> -BIG, f -> +BIG via the
    mbar row of xp; o captured at the freeze step via PE transpose +
    one fused scalar_tensor_tensor (o_acc = o^T_t * e_t + o_acc).
  - Decoder feedback folded: W_comb = Whh + Wih_dec @ out_W.
  - y = out_W @ h + out_b deferred entirely to a batched end-phase GEMM over
    h^T tiles dumped to DRAM each step (DMA engines are otherwise idle).
"""

import numpy as np
import ml_dtypes
from contextlib import ExitStack

import concourse.bass as bass
import concourse.bacc as bacc
import concourse.mybir as mybir
import concourse.tile as tile
from concourse.tile import add_dep_helper
from concourse.bass_utils import run_bass_kernel_spmd

B, T, D, H = 512, 512, 64, 256
G4 = 4 * H  # 1024
NCORES = 8
BL = B // NCORES  # 64
TDEC = T - 1      # 511 decoder steps
BIG = 30000.0
F32 = mybir.dt.float32
BF16 = mybir.dt.bfloat16
BF = ml_dtypes.bfloat16

_PROGRAM = None
LAST_RESULTS = None

# chunk order on the 512 free cols: [i0 i1 f0 f1 o0 o1 g0 g1]
# torch gate rows: i=[0,256) f=[256,512) g=[512,768) o=[768,1024)
CHUNK_ROWS = [(0, 128), (128, 256), (256, 384), (384, 512),
              (768, 896), (896, 1024), (512, 640), (640, 768)]

Sig = mybir.ActivationFunctionType.Sigmoid
Tanh = mybir.ActivationFunctionType.Tanh
Ident = mybir.ActivationFunctionType.Identity
MUL = mybir.AluOpType.mult
ADD = mybir.AluOpType.add


def build_program(t_enc=T, t_dec=TDEC, debug=False):
    nc = bacc.Bacc(None, target_bir_lowering=False)
    f = F32
    if debug:
        gdbg_d = nc.dram_tensor("gdbg", [128, 512], F32, kind="ExternalOutput")
        cdbg_d = nc.dram_tensor("cdbg", [128, 2, BL], F32, kind="ExternalOutput")
        hdbg_d = nc.dram_tensor("hdbg", [128, 2, BL], BF16, kind="ExternalOutput")
        odbg_d = nc.dram_tensor("odbg", [128, 128], BF16, kind="ExternalOutput")
        hbdbg_d = nc.dram_tensor("hbdbg", [128, 2, BL], BF16, kind="ExternalOutput")
    xp_d = nc.dram_tensor("xp", [t_enc, 66, BL], BF16, kind="ExternalInput")
    x0p_d = nc.dram_tensor("x0p", [66, BL], BF16, kind="ExternalInput")
    wxenc_d = nc.dram_tensor("wxenc", [66, 8, 128], BF16, kind="ExternalInput")
    wxdec_d = nc.dram_tensor("wxdec", [66, 8, 128], BF16, kind="ExternalInput")
    whhenc_d = nc.dram_tensor("whhenc", [128, 2, 8, 128], BF16, kind="ExternalInput")
    whhdec_d = nc.dram_tensor("whhdec", [128, 2, 8, 128], BF16, kind="ExternalInput")
    wcomb_d = nc.dram_tensor("wcomb", [128, 2, 8, 128], BF16, kind="ExternalInput")
    biasA_d = nc.dram_tensor("biasA", [4, 3, 128], BF16, kind="ExternalInput")
    biasB_d = nc.dram_tensor("biasB", [2, 3, 128], BF16, kind="ExternalInput")
    biasC_d = nc.dram_tensor("biasC", [2, 3, 128], BF16, kind="ExternalInput")
    obT_d = nc.dram_tensor("obT", [1, D], BF16, kind="ExternalInput")
    onesy_d = nc.dram_tensor("onesy", [1, 512], BF16, kind="ExternalInput")
    blockones_d = nc.dram_tensor("blockones", [8, 512], BF16, kind="ExternalInput")
    ident_d = nc.dram_tensor("ident", [128, 128], BF16, kind="ExternalInput")
    edup_d = nc.dram_tensor("edup", [128, t_enc], F32, kind="ExternalInput")
    outwT_d = nc.dram_tensor("outwT", [128, 2, D], BF16, kind="ExternalInput")
    outb_d = nc.dram_tensor("outb", [D, 1], F32, kind="ExternalInput")
    hdump_d = nc.dram_tensor("hdump", [128, t_dec, 2, BL], BF16, kind="Internal")
    yt_d = nc.dram_tensor("yt", [t_dec + 1, D, BL], F32, kind="ExternalOutput")

    with ExitStack() as ctx:
        tc = ctx.enter_context(tile.TileContext(nc))
        singles = ctx.enter_context(tc.tile_pool(name="singles", bufs=1))
        xpool = ctx.enter_context(tc.tile_pool(name="xpool", bufs=6))
        work = ctx.enter_context(tc.tile_pool(name="work", bufs=3))
        hpool = ctx.enter_context(tc.tile_pool(name="hpool", bufs=2))
        cpool = ctx.enter_context(tc.tile_pool(name="cpool", bufs=2))
        oap = ctx.enter_context(tc.tile_pool(name="oap", bufs=2))
        ybig = ctx.enter_context(tc.tile_pool(name="ybig", bufs=2))
        gpool = ctx.enter_context(
            tc.tile_pool(name="gpool", bufs=2, space=bass.MemorySpace.PSUM))
        tpp = ctx.enter_context(
            tc.tile_pool(name="tpp", bufs=1, space=bass.MemorySpace.PSUM))
        ypsum = ctx.enter_context(
            tc.tile_pool(name="ypsum", bufs=1, space=bass.MemorySpace.PSUM))

        # ---- persistent constants ----
        s_wxenc = singles.tile([66, 8, 128], BF16)
        nc.sync.dma_start(s_wxenc, wxenc_d[:, :, :])
        s_wxdec = singles.tile([66, 8, 128], BF16)
        nc.sync.dma_start(s_wxdec, wxdec_d[:, :, :])
        s_whhenc = singles.tile([128, 2, 8, 128], BF16)
        nc.sync.dma_start(s_whhenc, whhenc_d[:, :, :, :])
        s_whhdec = singles.tile([128, 2, 8, 128], BF16)
        nc.sync.dma_start(s_whhdec, whhdec_d[:, :, :, :])
        s_wcomb = singles.tile([128, 2, 8, 128], BF16)
        nc.sync.dma_start(s_wcomb, wcomb_d[:, :, :, :])
        s_biasA = singles.tile([4, 3, 128], BF16)
        nc.sync.dma_start(s_biasA, biasA_d[:, :, :])
        s_biasB = singles.tile([2, 3, 128], BF16)
        nc.sync.dma_start(s_biasB, biasB_d[:, :, :])
        s_biasC = singles.tile([2, 3, 128], BF16)
        nc.sync.dma_start(s_biasC, biasC_d[:, :, :])
        s_obT = singles.tile([1, D], BF16)
        nc.sync.dma_start(s_obT, obT_d[:, :])
        s_onesy = singles.tile([1, 512], BF16)
        nc.sync.dma_start(s_onesy, onesy_d[:, :])
        s_bonesA = singles.tile([4, 256], BF16)
        nc.sync.dma_start(s_bonesA, blockones_d[0:4, 0:256])
        s_bonesBC = singles.tile([2, 128], BF16)
        nc.sync.dma_start(s_bonesBC, blockones_d[4:6, 256:384])
        s_identb = singles.tile([128, 128], BF16)
        nc.sync.dma_start(s_identb, ident_d[:, :])
        s_edup = singles.tile([128, t_enc], F32)
        nc.sync.dma_start(s_edup, edup_d[:, :])
        s_outwT = singles.tile([128, 2, D], BF16)
        nc.sync.dma_start(s_outwT, outwT_d[:, :, :])
        s_outb = singles.tile([D, 1], f)
        nc.sync.dma_start(s_outb, outb_d[:, :])
        s_x0p = singles.tile([66, BL], BF16)
        nc.sync.dma_start(s_x0p, x0p_d[:, :])

        # ---- initial state ----
        c_prev = singles.tile([128, 2, BL], f, tag="c0")
        nc.vector.memset(c_prev, 0.0)
        hT_prev = singles.tile([128, 2, BL], BF16, tag="h0")
        nc.vector.memset(hT_prev, 0.0)
        o_acc = singles.tile([128, 128], BF16, tag="oacc0")
        nc.vector.memset(o_acc, 0.0)

        def chain(insts):
            for a, b in zip(insts[1:], insts[:-1]):
                add_dep_helper(a.ins, b.ins, sync=False, reason="pe-order")

        # gate chunk m -> (bank, col offset): A=i,f (m0-3), B=g (m6,7), C=o (m4,5)
        def bank_slice(psA, psB, psC, m):
            if m < 4:
                return psA[:, 64 * m:64 * m + 64]
            if m >= 6:
                return psB[:, 64 * (m - 6):64 * (m - 6) + 64]
            return psC[:, 64 * (m - 4):64 * (m - 4) + 64]

        def gate_mms(psA, psB, psC, whh, bset, xlhs=None, xrhs=None):
            """All matmuls of one step. Gates split across three PSUM banks
            so each ACT read waits only on its own bank's writers (PSUM
            bank-level write/read serialization is a hardware constraint).
            One start=True (bias) matmul opens each bank; bias + x MMs run in
            the PE-idle window; h MMs go bank-A-first. The explicit chain
            pins the scheduler to this PE order."""
            mms = [
                nc.tensor.matmul(psA, s_biasA[:, bset, :], s_bonesA,
                                 start=True, stop=False, skip_group_check=True),
                nc.tensor.matmul(psB, s_biasB[:, bset, :], s_bonesBC,
                                 start=True, stop=False, skip_group_check=True),
                nc.tensor.matmul(psC, s_biasC[:, bset, :], s_bonesBC,
                                 start=True, stop=False, skip_group_check=True),
            ]
            if xlhs is not None:
                for m in (0, 1, 2, 3, 6, 7, 4, 5):
                    mms.append(nc.tensor.matmul(bank_slice(psA, psB, psC, m),
                                                xlhs[:, m, :], xrhs,
                                                start=False, stop=False,
                                                skip_group_check=True))
            for ms in ((0, 1, 2, 3), (6, 7), (4, 5)):
                for k in (0, 1):
                    for m in ms:
                        mms.append(nc.tensor.matmul(
                            bank_slice(psA, psB, psC, m),
                            whh[:, k, m, :], hT_prev[:, k, :],
                            start=False, stop=(k == 1),
                            skip_group_check=True))
            chain(mms)
            return mms[-1]

        def cell(psA, psB, psC, enc_t):
            """LSTM cell elementwise phase. Updates c_prev/hT_prev (+o_acc)."""
            nonlocal c_prev, hT_prev, o_acc
            if_t = work.tile([128, 256], BF16, tag="ift")
            nc.scalar.activation(if_t, psA, Sig)
            g_t = work.tile([128, 128], BF16, tag="gt")
            nc.scalar.activation(g_t, psB, Tanh)
            o_t = work.tile([128, 128], BF16, tag="ot")
            nc.scalar.activation(o_t, psC, Sig)
            c_new = cpool.tile([128, 2, BL], f, tag="c")
            tct = work.tile([128, 2, BL], BF16, tag="tct")
            hT_new = hpool.tile([128, 2, BL], BF16, tag="hT")
            for k in (0, 1):
                sl = slice(64 * k, 64 * k + 64)
                fc = work.tile([128, BL], f, tag=f"fc{k}")
                nc.vector.tensor_mul(fc, if_t[:, 128 + 64 * k:192 + 64 * k],
                                     c_prev[:, k, :])
                ig = work.tile([128, BL], f, tag=f"ig{k}")
                nc.vector.tensor_mul(ig, if_t[:, sl], g_t[:, sl])
                nc.vector.tensor_add(c_new[:, k, :], fc, ig)
                nc.scalar.activation(tct[:, k, :], c_new[:, k, :], Tanh)
                nc.vector.tensor_mul(hT_new[:, k, :], o_t[:, sl], tct[:, k, :])
            if enc_t is not None:
                pending_o[0] = (o_t, enc_t)
            c_prev = c_new
            hT_prev = hT_new

        pending_o = [None]

        def flush_oacc(after=None):
            """Deferred o_acc capture: the PE transpose of step t's o is
            pinned after step t+1's matmuls so it never blocks the PE FIFO
            while waiting on sig_o."""
            nonlocal o_acc
            if pending_o[0] is None:
                return
            o_t, t = pending_o[0]
            pending_o[0] = None
            tp = tpp.tile([128, 128], BF16, tag="tp")
            tri = nc.tensor.transpose(tp, o_t, s_identb)
            if after is not None:
                add_dep_helper(tri.ins, after.ins, sync=False,
                               reason="defer transpose")
            o_acc2 = oap.tile([128, 128], BF16, tag="oacc")
            nc.vector.scalar_tensor_tensor(
                o_acc2, tp, s_edup[:, t:t + 1], o_acc, MUL, ADD)
            o_acc = o_acc2

        # ================= ENCODER =================
        for t in range(t_enc):
            xp_t = xpool.tile([66, BL], BF16, tag="xp")
            nc.sync.dma_start(xp_t, xp_d[t, :, :])
            psA = gpool.tile([128, 256], f, tag="gA")
            psB = gpool.tile([128, 128], f, tag="gB")
            psC = gpool.tile([128, 128], f, tag="gC")
            last_mm = gate_mms(psA, psB, psC, s_whhenc, 0,
                               xlhs=s_wxenc, xrhs=xp_t)
            flush_oacc(after=last_mm)
            if debug and t == 0:
                gcp = work.tile([128, 512], f, tag="gdbg")
                nc.vector.tensor_copy(gcp[:, 0:256], psA)
                nc.vector.tensor_copy(gcp[:, 384:512], psB)
                nc.vector.tensor_copy(gcp[:, 256:384], psC)
                nc.sync.dma_start(gdbg_d[:, :], gcp)
            cell(psA, psB, psC, t)

        if debug:
            nc.sync.dma_start(cdbg_d[:, :, :], c_prev)
            nc.sync.dma_start(hdbg_d[:, :, :], hT_prev)
            nc.sync.dma_start(odbg_d[:, :], o_acc)

        flush_oacc()

        # ===== boundary: hT_enc = o_sel^T * tanh(c_final) =====
        tce = work.tile([128, 2, BL], BF16, tag="tct")
        nc.scalar.activation(tce, c_prev, Tanh)
        tpe = tpp.tile([128, 128], BF16, tag="tp")
        nc.tensor.transpose(tpe, o_acc, s_identb)
        o_selT = work.tile([128, 128], BF16, tag="osel")
        nc.vector.tensor_copy(o_selT, tpe)
        hT_b = hpool.tile([128, 2, BL], BF16, tag="hT")
        for k in (0, 1):
            nc.vector.tensor_mul(hT_b[:, k, :], o_selT[:, 64 * k:64 * k + 64],
                                 tce[:, k, :])
        hT_prev = hT_b
        if debug:
            nc.sync.dma_start(hbdbg_d[:, :, :], hT_b)

        # ================= DECODER =================
        for j in range(t_dec):
            psA = gpool.tile([128, 256], f, tag="gA")
            psB = gpool.tile([128, 128], f, tag="gB")
            psC = gpool.tile([128, 128], f, tag="gC")
            if j == 0:
                gate_mms(psA, psB, psC, s_whhdec, 1,
                         xlhs=s_wxdec, xrhs=s_x0p)
            else:
                gate_mms(psA, psB, psC, s_wcomb, 2)
            cell(psA, psB, psC, None)
            nc.sync.dma_start(hdump_d[:, j, :, :], hT_prev)

        # ================= Y GEMM PHASE =================
        for s0 in range(0, t_dec, 64):
            n = min(64, t_dec - s0)
            hblk = ybig.tile([128, 64, 2, BL], BF16, tag="hblk")
            nc.sync.dma_start(hblk[:, 0:n, :, :], hdump_d[:, s0:s0 + n, :, :])
            for g0 in range(0, n, 8):
                cnt = min(8, n - g0)
                psy = ypsum.tile([D, 512], f, tag="psy")
                nc.tensor.matmul(psy, s_obT, s_onesy,
                                 start=True, stop=False, skip_group_check=True)
                for k in (0, 1):
                    for tl in range(cnt):
                        nc.tensor.matmul(psy[:, 64 * tl:64 * tl + 64],
                                         s_outwT[:, k, :], hblk[:, g0 + tl, k, :],
                                         start=False, stop=(k == 1),
                                         skip_group_check=True)
                y_sb = work.tile([D, 512], f, tag="ysb")
                nc.scalar.copy(y_sb[:, 0:64 * cnt], psy[:, 0:64 * cnt])
                for tl in range(cnt):
                    nc.sync.dma_start(yt_d[s0 + g0 + tl + 1, :, :],
                                      y_sb[:, 64 * tl:64 * tl + 64])

    nc.compile()
    return nc


def _prep_host(inputs, t_enc=T, t_dec=TDEC):
    """Build per-core in_maps from full inputs (numpy)."""
    x = np.asarray(inputs["input_tensor"], np.float32)
    tgt = np.asarray(inputs["target_tensor"], np.float32)
    lens = np.asarray(inputs["lens"]).astype(np.int64)

    eWih = np.asarray(inputs["enc_Wih"], np.float32)
    eWhh = np.asarray(inputs["enc_Whh"], np.float32)
    eb = (np.asarray(inputs["enc_bih"], np.float32)
          + np.asarray(inputs["enc_bhh"], np.float32))
    dWih = np.asarray(inputs["dec_Wih"], np.float32)
    dWhh = np.asarray(inputs["dec_Whh"], np.float32)
    db = (np.asarray(inputs["dec_bih"], np.float32)
          + np.asarray(inputs["dec_bhh"], np.float32))
    oW = np.asarray(inputs["out_W"], np.float32)
    ob = np.asarray(inputs["out_b"], np.float32)

    wcomb_full = dWhh + dWih @ oW          # [G4, H]
    bcomb = db + dWih @ ob                 # [G4]

    def chunked_x(W, freeze_big):
        # -> [66, 8, 128]: rows 0:64 x-weights^T, row 64 unused, row 65 freeze
        out = np.zeros((66, 8, 128), np.float32)
        for m, (r0, r1) in enumerate(CHUNK_ROWS):
            out[0:64, m, :] = W[r0:r1, :].T
            if freeze_big and m in (0, 1):
                out[65, m, :] = -BIG
            elif freeze_big and m in (2, 3):
                out[65, m, :] = BIG
        return out.astype(BF)

    def chunked_b(b):
        return np.stack([b[r0:r1] for (r0, r1) in CHUNK_ROWS])

    def chunked_h(W):
        # -> [128, 2, 8, 128]
        out = np.zeros((128, 2, 8, 128), np.float32)
        for m, (r0, r1) in enumerate(CHUNK_ROWS):
            for k in (0, 1):
                out[:, k, m, :] = W[r0:r1, 128 * k:128 * (k + 1)].T
        return out.astype(BF)

    wxenc = chunked_x(eWih, True)
    wxdec = chunked_x(dWih, False)
    whhenc = chunked_h(eWhh)
    whhdec = chunked_h(dWhh)
    wcomb = chunked_h(wcomb_full)
    # bias sets: 0=enc, 1=dec step0, 2=comb. Banks: A=m0-3, B=m6-7, C=m4-5.
    bsets = [chunked_b(eb), chunked_b(db), chunked_b(bcomb)]  # each [8, 128]
    biasA = np.stack([bs[0:4] for bs in bsets], 1).astype(BF)  # [4, 3, 128]
    biasB = np.stack([bs[6:8] for bs in bsets], 1).astype(BF)  # [2, 3, 128]
    biasC = np.stack([bs[4:6] for bs in bsets], 1).astype(BF)  # [2, 3, 128]
    obT = ob[None, :].astype(BF)
    onesy = np.ones((1, 512), np.float32).astype(BF)
    # blockones: rows 0-3 x cols 0-255 = 4x64 block-diag (bank A opener);
    # rows 4-5 x cols 256-383 = 2x64 block-diag (bank B/C opener)
    blockones = np.zeros((8, 512), np.float32)
    for m in range(4):
        blockones[m, 64 * m:64 * m + 64] = 1.0
    for m in range(2):
        blockones[4 + m, 256 + 64 * m:256 + 64 * m + 64] = 1.0
    blockones = blockones.astype(BF)
    ident = np.eye(128, dtype=np.float32).astype(BF)
    outwT = oW.T.reshape(2, 128, D).transpose(1, 0, 2).astype(BF).copy()
    outb = ob[:, None].astype(np.float32).copy()

    tt = np.arange(t_enc)[None, :]
    in_maps = []
    for c in range(NCORES):
        b0 = c * BL
        xs = x[b0:b0 + BL, :t_enc, :]                # [BL,t,D]
        xp = np.empty((t_enc, 66, BL), np.float32)
        xp[:, 0:D, :] = xs.transpose(1, 2, 0)
        xp[:, D, :] = 1.0
        lc = lens[b0:b0 + BL]
        mbar = (tt >= lc[:, None]).astype(np.float32)   # [BL,t]
        xp[:, D + 1, :] = mbar.T
        efreeze = (tt == (lc[:, None] - 1)).astype(np.float32)  # [BL,t]
        edup = np.concatenate([efreeze, efreeze], 0)    # [128,t]
        x0p = np.zeros((66, BL), np.float32)
        x0p[0:D, :] = tgt[b0:b0 + BL, 0, :].T
        x0p[D, :] = 1.0
        in_maps.append({
            "xp": np.ascontiguousarray(xp).astype(BF),
            "x0p": x0p.astype(BF),
            "wxenc": wxenc, "wxdec": wxdec,
            "whhenc": whhenc, "whhdec": whhdec, "wcomb": wcomb,
            "biasA": biasA, "biasB": biasB, "biasC": biasC,
            "obT": obT, "onesy": onesy,
            "blockones": blockones, "ident": ident,
            "edup": np.ascontiguousarray(edup),
            "outwT": outwT, "outb": outb,
        })
    return in_maps, lens


def kernel(**inputs) -> np.ndarray:
    global _PROGRAM, LAST_RESULTS
    if _PROGRAM is None:
        _PROGRAM = build_program()
    nc = _PROGRAM
    in_maps, lens = _prep_host(inputs)
    res = run_bass_kernel_spmd(nc, in_maps, core_ids=list(range(NCORES)))
    LAST_RESULTS = res
    out = np.zeros((B, T, D), np.float32)
    for c in range(NCORES):
        yt = res.results[c]["yt"]                      # [T, D, BL]
        out[c * BL:(c + 1) * BL] = yt.transpose(2, 0, 1)
    mask = (np.arange(T)[None, :] < lens[:, None])[:, :, None]
    out *= mask
    out[:, 0, :] = 0.0
    return out


# revision 44
# speedup vs baseline: 1.0831x; 1.0831x over previous
"""Trainium2 Bass kernel for the LSTM seq2seq autoencoder (layout B).

Strategy:
  - Data-parallel over batch: B=512 -> 64 rows per core on 8 cores.
  - Gates-on-partitions layout: gate preactivations live in one PSUM bank
    [128, 512] = 8 chunks x 64 batch cols, chunk order [i0 i1 f0 f1 o0 o1 g0 g1].
    Each step: 16 (h) + 8 (x, encoder) LDW+MM pairs of N=64 (FWL-eligible
    bf16 weights, LDWEIGHTS hidden behind matmuls) + one rank-8 bias matmul
    (decoder) whose rhs is a block-indicator [8, 512].
  - h^T [128, 2, 64] is produced directly by the DVE h-mul (no per-step
    transposes or PSUM->SBUF copies) and is the rhs of the next step's MMs.
  - Encoder length masking: c frozen by forcing i -> -BIG, f -> +BIG via the
    mbar row of xp; o captured at the freeze step via PE transpose +
    one fused scalar_tensor_tensor (o_acc = o^T_t * e_t + o_acc).
  - Decoder feedback folded: W_comb = Whh + Wih_dec @ out_W.
  - y = out_W @ h + out_b deferred entirely to a batched end-phase GEMM over
    h^T tiles dumped to DRAM each step (DMA engines are otherwise idle).
"""

import numpy as np
import ml_dtypes
from contextlib import ExitStack

import concourse.bass as bass
import concourse.bacc as bacc
import concourse.mybir as mybir
import concourse.tile as tile
from concourse.tile import add_dep_helper
from concourse.bass_utils import run_bass_kernel_spmd

B, T, D, H = 512, 512, 64, 256
G4 = 4 * H  # 1024
NCORES = 8
BL = B // NCORES  # 64
TDEC = T - 1      # 511 decoder steps
BIG = 30000.0
F32 = mybir.dt.float32
BF16 = mybir.dt.bfloat16
BF = ml_dtypes.bfloat16

_PROGRAM = None
LAST_RESULTS = None

# chunk order on the 512 free cols: [i0 i1 f0 f1 o0 o1 g0 g1]
# torch gate rows: i=[0,256) f=[256,512) g=[512,768) o=[768,1024)
CHUNK_ROWS = [(0, 128), (128, 256), (256, 384), (384, 512),
              (768, 896), (896, 1024), (512, 640), (640, 768)]

Sig = mybir.ActivationFunctionType.Sigmoid
Tanh = mybir.ActivationFunctionType.Tanh
Ident = mybir.ActivationFunctionType.Identity
MUL = mybir.AluOpType.mult
ADD = mybir.AluOpType.add


def build_program(t_enc=T, t_dec=TDEC, debug=False):
    nc = bacc.Bacc(None, target_bir_lowering=False)
    f = F32
    if debug:
        gdbg_d = nc.dram_tensor("gdbg", [128, 512], F32, kind="ExternalOutput")
        cdbg_d = nc.dram_tensor("cdbg", [128, 2, BL], F32, kind="ExternalOutput")
        hdbg_d = nc.dram_tensor("hdbg", [128, 2, BL], BF16, kind="ExternalOutput")
        odbg_d = nc.dram_tensor("odbg", [128, 128], BF16, kind="ExternalOutput")
        hbdbg_d = nc.dram_tensor("hbdbg", [128, 2, BL], BF16, kind="ExternalOutput")
    xp_d = nc.dram_tensor("xp", [t_enc, 66, BL], BF16, kind="ExternalInput")
    x0p_d = nc.dram_tensor("x0p", [66, BL], BF16, kind="ExternalInput")
    wxenc_d = nc.dram_tensor("wxenc", [66, 8, 128], BF16, kind="ExternalInput")
    wxdec_d = nc.dram_tensor("wxdec", [66, 8, 128], BF16, kind="ExternalInput")
    whhenc_d = nc.dram_tensor("whhenc", [128, 2, 8, 128], BF16, kind="ExternalInput")
    whhdec_d = nc.dram_tensor("whhdec", [128, 2, 8, 128], BF16, kind="ExternalInput")
    wcomb_d = nc.dram_tensor("wcomb", [128, 2, 8, 128], BF16, kind="ExternalInput")
    biasA_d = nc.dram_tensor("biasA", [4, 3, 128], BF16, kind="ExternalInput")
    biasB_d = nc.dram_tensor("biasB", [2, 3, 128], BF16, kind="ExternalInput")
    biasC_d = nc.dram_tensor("biasC", [2, 3, 128], BF16, kind="ExternalInput")
    obT_d = nc.dram_tensor("obT", [1, D], BF16, kind="ExternalInput")
    onesy_d = nc.dram_tensor("onesy", [1, 512], BF16, kind="ExternalInput")
    blockones_d = nc.dram_tensor("blockones", [8, 512], BF16, kind="ExternalInput")
    ident_d = nc.dram_tensor("ident", [128, 128], BF16, kind="ExternalInput")
    edup_d = nc.dram_tensor("edup", [128, t_enc], F32, kind="ExternalInput")
    outwT_d = nc.dram_tensor("outwT", [128, 2, D], BF16, kind="ExternalInput")
    outb_d = nc.dram_tensor("outb", [D, 1], F32, kind="ExternalInput")
    hdump_d = nc.dram_tensor("hdump", [128, t_dec, 2, BL], BF16, kind="Internal")
    yt_d = nc.dram_tensor("yt", [t_dec + 1, D, BL], F32, kind="ExternalOutput")

    with ExitStack() as ctx:
        tc = ctx.enter_context(tile.TileContext(nc))
        singles = ctx.enter_context(tc.tile_pool(name="singles", bufs=1))
        xpool = ctx.enter_context(tc.tile_pool(name="xpool", bufs=6))
        work = ctx.enter_context(tc.tile_pool(name="work", bufs=3))
        hpool = ctx.enter_context(tc.tile_pool(name="hpool", bufs=2))
        cpool = ctx.enter_context(tc.tile_pool(name="cpool", bufs=2))
        oap = ctx.enter_context(tc.tile_pool(name="oap", bufs=2))
        ybig = ctx.enter_context(tc.tile_pool(name="ybig", bufs=2))
        gpool = ctx.enter_context(
            tc.tile_pool(name="gpool", bufs=3, space=bass.MemorySpace.PSUM))
        gbc = ctx.enter_context(
            tc.tile_pool(name="gbc", bufs=2, space=bass.MemorySpace.PSUM))
        tpp = ctx.enter_context(
            tc.tile_pool(name="tpp", bufs=1, space=bass.MemorySpace.PSUM))

        # ---- persistent constants ----
        s_wxenc = singles.tile([66, 8, 128], BF16)
        nc.sync.dma_start(s_wxenc, wxenc_d[:, :, :])
        s_wxdec = singles.tile([66, 8, 128], BF16)
        nc.sync.dma_start(s_wxdec, wxdec_d[:, :, :])
        s_whhenc = singles.tile([128, 2, 8, 128], BF16)
        nc.sync.dma_start(s_whhenc, whhenc_d[:, :, :, :])
        s_whhdec = singles.tile([128, 2, 8, 128], BF16)
        nc.sync.dma_start(s_whhdec, whhdec_d[:, :, :, :])
        s_wcomb = singles.tile([128, 2, 8, 128], BF16)
        nc.sync.dma_start(s_wcomb, wcomb_d[:, :, :, :])
        s_biasA = singles.tile([4, 3, 128], BF16)
        nc.sync.dma_start(s_biasA, biasA_d[:, :, :])
        s_biasB = singles.tile([2, 3, 128], BF16)
        nc.sync.dma_start(s_biasB, biasB_d[:, :, :])
        s_biasC = singles.tile([2, 3, 128], BF16)
        nc.sync.dma_start(s_biasC, biasC_d[:, :, :])
        s_obT = singles.tile([1, D], BF16)
        nc.sync.dma_start(s_obT, obT_d[:, :])
        s_onesy = singles.tile([1, 512], BF16)
        nc.sync.dma_start(s_onesy, onesy_d[:, :])
        s_bonesA = singles.tile([4, 256], BF16)
        nc.sync.dma_start(s_bonesA, blockones_d[0:4, 0:256])
        s_bonesBC = singles.tile([2, 128], BF16)
        nc.sync.dma_start(s_bonesBC, blockones_d[4:6, 256:384])
        s_identb = singles.tile([128, 128], BF16)
        nc.sync.dma_start(s_identb, ident_d[:, :])
        s_edup = singles.tile([128, t_enc], F32)
        nc.sync.dma_start(s_edup, edup_d[:, :])
        s_outwT = singles.tile([128, 2, D], BF16)
        nc.sync.dma_start(s_outwT, outwT_d[:, :, :])
        s_outb = singles.tile([D, 1], f)
        nc.sync.dma_start(s_outb, outb_d[:, :])
        s_x0p = singles.tile([66, BL], BF16)
        nc.sync.dma_start(s_x0p, x0p_d[:, :])

        # ---- initial state ----
        c_prev = singles.tile([128, 2, BL], f, tag="c0")
        nc.vector.memset(c_prev, 0.0)
        hT_i0 = singles.tile([128, BL], BF16, tag="hi0")
        nc.vector.memset(hT_i0, 0.0)
        hT_i1 = singles.tile([128, BL], BF16, tag="hi1")
        nc.vector.memset(hT_i1, 0.0)
        hT_prev = (hT_i0, hT_i1)
        o_acc = singles.tile([128, 128], BF16, tag="oacc0")
        nc.vector.memset(o_acc, 0.0)

        def chain(insts):
            for a, b in zip(insts[1:], insts[:-1]):
                add_dep_helper(a.ins, b.ins, sync=False, reason="pe-order")

        # gate chunk m -> (bank, col offset): A=i,f (m0-3), B=g (m6,7), C=o (m4,5)
        def bank_slice(psA, psB, psC, m):
            if m < 4:
                return psA[:, 64 * m:64 * m + 64]
            if m >= 6:
                return psB[:, 64 * (m - 6):64 * (m - 6) + 64]
            return psC[:, 64 * (m - 4):64 * (m - 4) + 64]

        def gate_mms(psA, psB, psC, whh, bset, xlhs=None, xrhs=None):
            """All matmuls of one step. Gates split across three PSUM banks
            so each ACT read waits only on its own bank's writers (PSUM
            bank-level write/read serialization is a hardware constraint).
            One start=True (bias) matmul opens each bank; bias + x MMs run in
            the PE-idle window; h MMs go bank-A-first. The explicit chain
            pins the scheduler to this PE order."""
            mms = [
                nc.tensor.matmul(psA, s_biasA[:, bset, :], s_bonesA,
                                 start=True, stop=False, skip_group_check=True),
                nc.tensor.matmul(psB, s_biasB[:, bset, :], s_bonesBC,
                                 start=True, stop=False, skip_group_check=True),
                nc.tensor.matmul(psC, s_biasC[:, bset, :], s_bonesBC,
                                 start=True, stop=False, skip_group_check=True),
            ]
            if xlhs is not None:
                for m in (0, 1, 2, 3, 6, 7, 4, 5):
                    mms.append(nc.tensor.matmul(bank_slice(psA, psB, psC, m),
                                                xlhs[:, m, :], xrhs,
                                                start=False, stop=False,
                                                skip_group_check=True))
            # k0 group first (gated only by h0), then k1 group (h1); within
            # each group bank A first so sig_if's bank completes earliest
            for k in (0, 1):
                for m in (0, 1, 2, 3, 6, 7, 4, 5):
                    mms.append(nc.tensor.matmul(
                        bank_slice(psA, psB, psC, m),
                        whh[:, k, m, :], hT_prev[k],
                        start=False, stop=(k == 1),
                        skip_group_check=True))
            chain(mms)
            return mms[-1]

        def cell(psA, psB, psC, enc_t):
            """LSTM cell elementwise phase. Updates c_prev/hT_prev (+o_acc)."""
            nonlocal c_prev, hT_prev, o_acc
            if_t = work.tile([128, 256], BF16, tag="ift")
            nc.scalar.activation(if_t, psA, Sig)
            g_t = work.tile([128, 128], BF16, tag="gt")
            nc.scalar.activation(g_t, psB, Tanh)
            o_t = work.tile([128, 128], BF16, tag="ot")
            nc.scalar.activation(o_t, psC, Sig)
            c_new = cpool.tile([128, 2, BL], f, tag="c")
            tct = work.tile([128, 2, BL], BF16, tag="tct")
            hT_new = (hpool.tile([128, BL], BF16, tag="hT0", name="hT0"),
                      hpool.tile([128, BL], BF16, tag="hT1", name="hT1"))
            for k in (0, 1):
                sl = slice(64 * k, 64 * k + 64)
                fc = work.tile([128, BL], f, tag=f"fc{k}")
                nc.vector.tensor_mul(fc, if_t[:, 128 + 64 * k:192 + 64 * k],
                                     c_prev[:, k, :])
                ig = work.tile([128, BL], f, tag=f"ig{k}")
                nc.vector.tensor_mul(ig, if_t[:, sl], g_t[:, sl])
                nc.vector.tensor_add(c_new[:, k, :], fc, ig)
                nc.scalar.activation(tct[:, k, :], c_new[:, k, :], Tanh)
                nc.vector.tensor_mul(hT_new[k], o_t[:, sl], tct[:, k, :])
            if enc_t is not None:
                pending_o[0] = (o_t, enc_t)
            c_prev = c_new
            hT_prev = hT_new

        pending_o = [None]

        def flush_oacc(after=None):
            """Deferred o_acc capture: the PE transpose of step t's o is
            pinned after step t+1's matmuls so it never blocks the PE FIFO
            while waiting on sig_o."""
            nonlocal o_acc
            if pending_o[0] is None:
                return
            o_t, t = pending_o[0]
            pending_o[0] = None
            tp = tpp.tile([128, 128], BF16, tag="tp")
            tri = nc.tensor.transpose(tp, o_t, s_identb)
            if after is not None:
                add_dep_helper(tri.ins, after.ins, sync=False,
                               reason="defer transpose")
            o_acc2 = oap.tile([128, 128], BF16, tag="oacc")
            nc.vector.scalar_tensor_tensor(
                o_acc2, tp, s_edup[:, t:t + 1], o_acc, MUL, ADD)
            o_acc = o_acc2

        # ================= ENCODER =================
        for t in range(t_enc):
            xp_t = xpool.tile([66, BL], BF16, tag="xp")
            nc.sync.dma_start(xp_t, xp_d[t, :, :])
            psA = gpool.tile([128, 256], f, tag="gA")
            psB = gbc.tile([128, 128], f, tag="gB")
            psC = gbc.tile([128, 128], f, tag="gC")
            last_mm = gate_mms(psA, psB, psC, s_whhenc, 0,
                               xlhs=s_wxenc, xrhs=xp_t)
            flush_oacc(after=last_mm)
            if debug and t == 0:
                gcp = work.tile([128, 512], f, tag="gdbg")
                nc.vector.tensor_copy(gcp[:, 0:256], psA)
                nc.vector.tensor_copy(gcp[:, 384:512], psB)
                nc.vector.tensor_copy(gcp[:, 256:384], psC)
                nc.sync.dma_start(gdbg_d[:, :], gcp)
            cell(psA, psB, psC, t)

        if debug:
            nc.sync.dma_start(cdbg_d[:, :, :], c_prev)
            nc.sync.dma_start(hdbg_d[:, 0, :], hT_prev[0])
            nc.sync.dma_start(hdbg_d[:, 1, :], hT_prev[1])
            nc.sync.dma_start(odbg_d[:, :], o_acc)

        flush_oacc()

        # ===== boundary: hT_enc = o_sel^T * tanh(c_final) =====
        tce = work.tile([128, 2, BL], BF16, tag="tct")
        nc.scalar.activation(tce, c_prev, Tanh)
        tpe = tpp.tile([128, 128], BF16, tag="tp")
        nc.tensor.transpose(tpe, o_acc, s_identb)
        o_selT = work.tile([128, 128], BF16, tag="osel")
        nc.vector.tensor_copy(o_selT, tpe)
        hT_b = (hpool.tile([128, BL], BF16, tag="hT0", name="hTb0"),
                hpool.tile([128, BL], BF16, tag="hT1", name="hTb1"))
        for k in (0, 1):
            nc.vector.tensor_mul(hT_b[k], o_selT[:, 64 * k:64 * k + 64],
                                 tce[:, k, :])
        hT_prev = hT_b
        if debug:
            nc.sync.dma_start(hbdbg_d[:, 0, :], hT_b[0])
            nc.sync.dma_start(hbdbg_d[:, 1, :], hT_b[1])

        # ================= DECODER =================
        for j in range(t_dec):
            psA = gpool.tile([128, 256], f, tag="gA")
            psB = gbc.tile([128, 128], f, tag="gB")
            psC = gbc.tile([128, 128], f, tag="gC")
            if j == 0:
                gate_mms(psA, psB, psC, s_whhdec, 1,
                         xlhs=s_wxdec, xrhs=s_x0p)
            else:
                gate_mms(psA, psB, psC, s_wcomb, 2)
            cell(psA, psB, psC, None)
            nc.sync.dma_start(hdump_d[:, j, 0, :], hT_prev[0])
            nc.sync.dma_start(hdump_d[:, j, 1, :], hT_prev[1])

        # ================= Y GEMM PHASE =================
        for s0 in range(0, t_dec, 64):
            n = min(64, t_dec - s0)
            hblk = ybig.tile([128, 64, 2, BL], BF16, tag="hblk")
            nc.sync.dma_start(hblk[:, 0:n, :, :], hdump_d[:, s0:s0 + n, :, :])
            for g0 in range(0, n, 8):
                cnt = min(8, n - g0)
                psy = gpool.tile([D, 512], f, tag="gA")
                nc.tensor.matmul(psy, s_obT, s_onesy,
                                 start=True, stop=False, skip_group_check=True)
                for k in (0, 1):
                    for tl in range(cnt):
                        nc.tensor.matmul(psy[:, 64 * tl:64 * tl + 64],
                                         s_outwT[:, k, :], hblk[:, g0 + tl, k, :],
                                         start=False, stop=(k == 1),
                                         skip_group_check=True)
                y_sb = work.tile([D, 512], f, tag="ysb")
                nc.scalar.copy(y_sb[:, 0:64 * cnt], psy[:, 0:64 * cnt])
                for tl in range(cnt):
                    nc.sync.dma_start(yt_d[s0 + g0 + tl + 1, :, :],
                                      y_sb[:, 64 * tl:64 * tl + 64])

    nc.compile()
    return nc


def _prep_host(inputs, t_enc=T, t_dec=TDEC):
    """Build per-core in_maps from full inputs (numpy)."""
    x = np.asarray(inputs["input_tensor"], np.float32)
    tgt = np.asarray(inputs["target_tensor"], np.float32)
    lens = np.asarray(inputs["lens"]).astype(np.int64)

    eWih = np.asarray(inputs["enc_Wih"], np.float32)
    eWhh = np.asarray(inputs["enc_Whh"], np.float32)
    eb = (np.asarray(inputs["enc_bih"], np.float32)
          + np.asarray(inputs["enc_bhh"], np.float32))
    dWih = np.asarray(inputs["dec_Wih"], np.float32)
    dWhh = np.asarray(inputs["dec_Whh"], np.float32)
    db = (np.asarray(inputs["dec_bih"], np.float32)
          + np.asarray(inputs["dec_bhh"], np.float32))
    oW = np.asarray(inputs["out_W"], np.float32)
    ob = np.asarray(inputs["out_b"], np.float32)

    wcomb_full = dWhh + dWih @ oW          # [G4, H]
    bcomb = db + dWih @ ob                 # [G4]

    def chunked_x(W, freeze_big):
        # -> [66, 8, 128]: rows 0:64 x-weights^T, row 64 unused, row 65 freeze
        out = np.zeros((66, 8, 128), np.float32)
        for m, (r0, r1) in enumerate(CHUNK_ROWS):
            out[0:64, m, :] = W[r0:r1, :].T
            if freeze_big and m in (0, 1):
                out[65, m, :] = -BIG
            elif freeze_big and m in (2, 3):
                out[65, m, :] = BIG
        return out.astype(BF)

    def chunked_b(b):
        return np.stack([b[r0:r1] for (r0, r1) in CHUNK_ROWS])

    def chunked_h(W):
        # -> [128, 2, 8, 128]
        out = np.zeros((128, 2, 8, 128), np.float32)
        for m, (r0, r1) in enumerate(CHUNK_ROWS):
            for k in (0, 1):
                out[:, k, m, :] = W[r0:r1, 128 * k:128 * (k + 1)].T
        return out.astype(BF)

    wxenc = chunked_x(eWih, True)
    wxdec = chunked_x(dWih, False)
    whhenc = chunked_h(eWhh)
    whhdec = chunked_h(dWhh)
    wcomb = chunked_h(wcomb_full)
    # bias sets: 0=enc, 1=dec step0, 2=comb. Banks: A=m0-3, B=m6-7, C=m4-5.
    bsets = [chunked_b(eb), chunked_b(db), chunked_b(bcomb)]  # each [8, 128]
    biasA = np.stack([bs[0:4] for bs in bsets], 1).astype(BF)  # [4, 3, 128]
    biasB = np.stack([bs[6:8] for bs in bsets], 1).astype(BF)  # [2, 3, 128]
    biasC = np.stack([bs[4:6] for bs in bsets], 1).astype(BF)  # [2, 3, 128]
    obT = ob[None, :].astype(BF)
    onesy = np.ones((1, 512), np.float32).astype(BF)
    # blockones: rows 0-3 x cols 0-255 = 4x64 block-diag (bank A opener);
    # rows 4-5 x cols 256-383 = 2x64 block-diag (bank B/C opener)
    blockones = np.zeros((8, 512), np.float32)
    for m in range(4):
        blockones[m, 64 * m:64 * m + 64] = 1.0
    for m in range(2):
        blockones[4 + m, 256 + 64 * m:256 + 64 * m + 64] = 1.0
    blockones = blockones.astype(BF)
    ident = np.eye(128, dtype=np.float32).astype(BF)
    outwT = oW.T.reshape(2, 128, D).transpose(1, 0, 2).astype(BF).copy()
    outb = ob[:, None].astype(np.float32).copy()

    tt = np.arange(t_enc)[None, :]
    in_maps = []
    for c in range(NCORES):
        b0 = c * BL
        xs = x[b0:b0 + BL, :t_enc, :]                # [BL,t,D]
        xp = np.empty((t_enc, 66, BL), np.float32)
        xp[:, 0:D, :] = xs.transpose(1, 2, 0)
        xp[:, D, :] = 1.0
        lc = lens[b0:b0 + BL]
        mbar = (tt >= lc[:, None]).astype(np.float32)   # [BL,t]
        xp[:, D + 1, :] = mbar.T
        efreeze = (tt == (lc[:, None] - 1)).astype(np.float32)  # [BL,t]
        edup = np.concatenate([efreeze, efreeze], 0)    # [128,t]
        x0p = np.zeros((66, BL), np.float32)
        x0p[0:D, :] = tgt[b0:b0 + BL, 0, :].T
        x0p[D, :] = 1.0
        in_maps.append({
            "xp": np.ascontiguousarray(xp).astype(BF),
            "x0p": x0p.astype(BF),
            "wxenc": wxenc, "wxdec": wxdec,
            "whhenc": whhenc, "whhdec": whhdec, "wcomb": wcomb,
            "biasA": biasA, "biasB": biasB, "biasC": biasC,
            "obT": obT, "onesy": onesy,
            "blockones": blockones, "ident": ident,
            "edup": np.ascontiguousarray(edup),
            "outwT": outwT, "outb": outb,
        })
    return in_maps, lens


def kernel(**inputs) -> np.ndarray:
    global _PROGRAM, LAST_RESULTS
    if _PROGRAM is None:
        _PROGRAM = build_program()
    nc = _PROGRAM
    in_maps, lens = _prep_host(inputs)
    res = run_bass_kernel_spmd(nc, in_maps, core_ids=list(range(NCORES)))
    LAST_RESULTS = res
    out = np.zeros((B, T, D), np.float32)
    for c in range(NCORES):
        yt = res.results[c]["yt"]                      # [T, D, BL]
        out[c * BL:(c + 1) * BL] = yt.transpose(2, 0, 1)
    mask = (np.arange(T)[None, :] < lens[:, None])[:, :, None]
    out *= mask
    out[:, 0, :] = 0.0
    return out


# revision 45
# speedup vs baseline: 1.1196x; 1.0337x over previous
"""Trainium2 Bass kernel for the LSTM seq2seq autoencoder (layout B).

Strategy:
  - Data-parallel over batch: B=512 -> 64 rows per core on 8 cores.
  - Gates-on-partitions layout: gate preactivations live in one PSUM bank
    [128, 512] = 8 chunks x 64 batch cols, chunk order [i0 i1 f0 f1 o0 o1 g0 g1].
    Each step: 16 (h) + 8 (x, encoder) LDW+MM pairs of N=64 (FWL-eligible
    bf16 weights, LDWEIGHTS hidden behind matmuls) + one rank-8 bias matmul
    (decoder) whose rhs is a block-indicator [8, 512].
  - h^T [128, 2, 64] is produced directly by the DVE h-mul (no per-step
    transposes or PSUM->SBUF copies) and is the rhs of the next step's MMs.
  - Encoder length masking: c frozen by forcing i -> -BIG, f -> +BIG via the
    mbar row of xp; o captured at the freeze step via PE transpose +
    one fused scalar_tensor_tensor (o_acc = o^T_t * e_t + o_acc).
  - Decoder feedback folded: W_comb = Whh + Wih_dec @ out_W.
  - y = out_W @ h + out_b deferred entirely to a batched end-phase GEMM over
    h^T tiles dumped to DRAM each step (DMA engines are otherwise idle).
"""

import numpy as np
import ml_dtypes
from contextlib import ExitStack

import concourse.bass as bass
import concourse.bacc as bacc
import concourse.mybir as mybir
import concourse.tile as tile
from concourse.tile import add_dep_helper
from concourse.bass_utils import run_bass_kernel_spmd

B, T, D, H = 512, 512, 64, 256
G4 = 4 * H  # 1024
NCORES = 8
BL = B // NCORES  # 64
TDEC = T - 1      # 511 decoder steps
BIG = 30000.0
F32 = mybir.dt.float32
BF16 = mybir.dt.bfloat16
BF = ml_dtypes.bfloat16

_PROGRAM = None
LAST_RESULTS = None

# chunk order on the 512 free cols: [i0 i1 f0 f1 o0 o1 g0 g1]
# torch gate rows: i=[0,256) f=[256,512) g=[512,768) o=[768,1024)
CHUNK_ROWS = [(0, 128), (128, 256), (256, 384), (384, 512),
              (768, 896), (896, 1024), (512, 640), (640, 768)]

Sig = mybir.ActivationFunctionType.Sigmoid
Tanh = mybir.ActivationFunctionType.Tanh
Ident = mybir.ActivationFunctionType.Identity
MUL = mybir.AluOpType.mult
ADD = mybir.AluOpType.add


def build_program(t_enc=T, t_dec=TDEC, debug=False):
    nc = bacc.Bacc(None, target_bir_lowering=False)
    f = F32
    if debug:
        gdbg_d = nc.dram_tensor("gdbg", [128, 512], F32, kind="ExternalOutput")
        cdbg_d = nc.dram_tensor("cdbg", [128, 2, BL], F32, kind="ExternalOutput")
        hdbg_d = nc.dram_tensor("hdbg", [128, 2, BL], BF16, kind="ExternalOutput")
        odbg_d = nc.dram_tensor("odbg", [128, 128], BF16, kind="ExternalOutput")
        hbdbg_d = nc.dram_tensor("hbdbg", [128, 2, BL], BF16, kind="ExternalOutput")
    xp_d = nc.dram_tensor("xp", [t_enc, 66, BL], BF16, kind="ExternalInput")
    x0p_d = nc.dram_tensor("x0p", [66, BL], BF16, kind="ExternalInput")
    wxenc_d = nc.dram_tensor("wxenc", [66, 8, 128], BF16, kind="ExternalInput")
    wxdec_d = nc.dram_tensor("wxdec", [66, 8, 128], BF16, kind="ExternalInput")
    whhenc_d = nc.dram_tensor("whhenc", [128, 2, 8, 128], BF16, kind="ExternalInput")
    whhdec_d = nc.dram_tensor("whhdec", [128, 2, 8, 128], BF16, kind="ExternalInput")
    wcomb_d = nc.dram_tensor("wcomb", [128, 2, 8, 128], BF16, kind="ExternalInput")
    biasA_d = nc.dram_tensor("biasA", [4, 3, 128], BF16, kind="ExternalInput")
    biasB_d = nc.dram_tensor("biasB", [2, 3, 128], BF16, kind="ExternalInput")
    biasC_d = nc.dram_tensor("biasC", [2, 3, 128], BF16, kind="ExternalInput")
    obT_d = nc.dram_tensor("obT", [1, D], BF16, kind="ExternalInput")
    onesy_d = nc.dram_tensor("onesy", [1, 512], BF16, kind="ExternalInput")
    blockones_d = nc.dram_tensor("blockones", [8, 512], BF16, kind="ExternalInput")
    ident_d = nc.dram_tensor("ident", [128, 128], BF16, kind="ExternalInput")
    edup_d = nc.dram_tensor("edup", [128, t_enc], F32, kind="ExternalInput")
    outwT_d = nc.dram_tensor("outwT", [128, 2, D], BF16, kind="ExternalInput")
    outb_d = nc.dram_tensor("outb", [D, 1], F32, kind="ExternalInput")
    yt_d = nc.dram_tensor("yt", [D, t_dec + 1, BL], F32, kind="ExternalOutput")

    with ExitStack() as ctx:
        tc = ctx.enter_context(tile.TileContext(nc))
        singles = ctx.enter_context(tc.tile_pool(name="singles", bufs=1))
        xpool = ctx.enter_context(tc.tile_pool(name="xpool", bufs=6))
        work = ctx.enter_context(tc.tile_pool(name="work", bufs=3))
        hpool = ctx.enter_context(tc.tile_pool(name="hpool", bufs=2))
        cpool = ctx.enter_context(tc.tile_pool(name="cpool", bufs=2))
        oap = ctx.enter_context(tc.tile_pool(name="oap", bufs=2))
        gpool = ctx.enter_context(
            tc.tile_pool(name="gpool", bufs=3, space=bass.MemorySpace.PSUM))
        gbc = ctx.enter_context(
            tc.tile_pool(name="gbc", bufs=2, space=bass.MemorySpace.PSUM))
        tpp = ctx.enter_context(
            tc.tile_pool(name="tpp", bufs=1, space=bass.MemorySpace.PSUM))

        # ---- persistent constants ----
        s_wxenc = singles.tile([66, 8, 128], BF16)
        nc.sync.dma_start(s_wxenc, wxenc_d[:, :, :])
        s_wxdec = singles.tile([66, 8, 128], BF16)
        nc.sync.dma_start(s_wxdec, wxdec_d[:, :, :])
        s_whhenc = singles.tile([128, 2, 8, 128], BF16)
        nc.sync.dma_start(s_whhenc, whhenc_d[:, :, :, :])
        s_whhdec = singles.tile([128, 2, 8, 128], BF16)
        nc.sync.dma_start(s_whhdec, whhdec_d[:, :, :, :])
        s_wcomb = singles.tile([128, 2, 8, 128], BF16)
        nc.sync.dma_start(s_wcomb, wcomb_d[:, :, :, :])
        s_biasA = singles.tile([4, 3, 128], BF16)
        nc.sync.dma_start(s_biasA, biasA_d[:, :, :])
        s_biasB = singles.tile([2, 3, 128], BF16)
        nc.sync.dma_start(s_biasB, biasB_d[:, :, :])
        s_biasC = singles.tile([2, 3, 128], BF16)
        nc.sync.dma_start(s_biasC, biasC_d[:, :, :])
        s_obT = singles.tile([1, D], BF16)
        nc.sync.dma_start(s_obT, obT_d[:, :])
        s_onesy = singles.tile([1, 512], BF16)
        nc.sync.dma_start(s_onesy, onesy_d[:, :])
        s_bonesA = singles.tile([4, 256], BF16)
        nc.sync.dma_start(s_bonesA, blockones_d[0:4, 0:256])
        s_bonesBC = singles.tile([2, 128], BF16)
        nc.sync.dma_start(s_bonesBC, blockones_d[4:6, 256:384])
        s_identb = singles.tile([128, 128], BF16)
        nc.sync.dma_start(s_identb, ident_d[:, :])
        s_edup = singles.tile([128, t_enc], F32)
        nc.sync.dma_start(s_edup, edup_d[:, :])
        s_outwT = singles.tile([128, 2, D], BF16)
        nc.sync.dma_start(s_outwT, outwT_d[:, :, :])
        s_outb = singles.tile([D, 1], f)
        nc.sync.dma_start(s_outb, outb_d[:, :])
        s_x0p = singles.tile([66, BL], BF16)
        nc.sync.dma_start(s_x0p, x0p_d[:, :])

        # ---- initial state ----
        c_prev = singles.tile([128, 2, BL], f, tag="c0")
        nc.vector.memset(c_prev, 0.0)
        hT_i0 = singles.tile([128, BL], BF16, tag="hi0")
        nc.vector.memset(hT_i0, 0.0)
        hT_i1 = singles.tile([128, BL], BF16, tag="hi1")
        nc.vector.memset(hT_i1, 0.0)
        hT_prev = (hT_i0, hT_i1)
        o_acc = singles.tile([128, 128], BF16, tag="oacc0")
        nc.vector.memset(o_acc, 0.0)

        def chain(insts):
            for a, b in zip(insts[1:], insts[:-1]):
                add_dep_helper(a.ins, b.ins, sync=False, reason="pe-order")

        # gate chunk m -> (bank, col offset): A=i,f (m0-3), B=g (m6,7), C=o (m4,5)
        def bank_slice(psA, psB, psC, m):
            if m < 4:
                return psA[:, 64 * m:64 * m + 64]
            if m >= 6:
                return psB[:, 64 * (m - 6):64 * (m - 6) + 64]
            return psC[:, 64 * (m - 4):64 * (m - 4) + 64]

        def gate_mms(psA, psB, psC, whh, bset, xlhs=None, xrhs=None):
            """All matmuls of one step. Gates split across three PSUM banks
            so each ACT read waits only on its own bank's writers (PSUM
            bank-level write/read serialization is a hardware constraint).
            One start=True (bias) matmul opens each bank; bias + x MMs run in
            the PE-idle window; h MMs go bank-A-first. The explicit chain
            pins the scheduler to this PE order."""
            mms = [
                nc.tensor.matmul(psA, s_biasA[:, bset, :], s_bonesA,
                                 start=True, stop=False, skip_group_check=True),
                nc.tensor.matmul(psB, s_biasB[:, bset, :], s_bonesBC,
                                 start=True, stop=False, skip_group_check=True),
                nc.tensor.matmul(psC, s_biasC[:, bset, :], s_bonesBC,
                                 start=True, stop=False, skip_group_check=True),
            ]
            if xlhs is not None:
                for m in (0, 1, 2, 3, 6, 7, 4, 5):
                    mms.append(nc.tensor.matmul(bank_slice(psA, psB, psC, m),
                                                xlhs[:, m, :], xrhs,
                                                start=False, stop=False,
                                                skip_group_check=True))
            # k0 group first (gated only by h0), then k1 group (h1); within
            # each group bank A first so sig_if's bank completes earliest
            for k in (0, 1):
                for m in (0, 1, 2, 3, 6, 7, 4, 5):
                    mms.append(nc.tensor.matmul(
                        bank_slice(psA, psB, psC, m),
                        whh[:, k, m, :], hT_prev[k],
                        start=False, stop=(k == 1),
                        skip_group_check=True))
            chain(mms)
            return mms[-1]

        def cell(psA, psB, psC, enc_t):
            """LSTM cell elementwise phase. Updates c_prev/hT_prev (+o_acc)."""
            nonlocal c_prev, hT_prev, o_acc
            if_t = work.tile([128, 256], BF16, tag="ift")
            nc.scalar.activation(if_t, psA, Sig)
            g_t = work.tile([128, 128], BF16, tag="gt")
            nc.scalar.activation(g_t, psB, Tanh)
            o_t = work.tile([128, 128], BF16, tag="ot")
            nc.scalar.activation(o_t, psC, Sig)
            c_new = cpool.tile([128, 2, BL], f, tag="c")
            tct = work.tile([128, 2, BL], BF16, tag="tct")
            hT_new = (hpool.tile([128, BL], BF16, tag="hT0", name="hT0"),
                      hpool.tile([128, BL], BF16, tag="hT1", name="hT1"))
            for k in (0, 1):
                sl = slice(64 * k, 64 * k + 64)
                fc = work.tile([128, BL], f, tag=f"fc{k}")
                nc.vector.tensor_mul(fc, if_t[:, 128 + 64 * k:192 + 64 * k],
                                     c_prev[:, k, :])
                ig = work.tile([128, BL], f, tag=f"ig{k}")
                nc.vector.tensor_mul(ig, if_t[:, sl], g_t[:, sl])
                nc.vector.tensor_add(c_new[:, k, :], fc, ig)
                nc.scalar.activation(tct[:, k, :], c_new[:, k, :], Tanh)
                nc.vector.tensor_mul(hT_new[k], o_t[:, sl], tct[:, k, :])
            if enc_t is not None:
                pending_o[0] = (o_t, enc_t)
            c_prev = c_new
            hT_prev = hT_new

        pending_o = [None]

        def flush_oacc(after=None):
            """Deferred o_acc capture: the PE transpose of step t's o is
            pinned after step t+1's matmuls so it never blocks the PE FIFO
            while waiting on sig_o."""
            nonlocal o_acc
            if pending_o[0] is None:
                return
            o_t, t = pending_o[0]
            pending_o[0] = None
            tp = tpp.tile([128, 128], BF16, tag="tp")
            tri = nc.tensor.transpose(tp, o_t, s_identb)
            if after is not None:
                add_dep_helper(tri.ins, after.ins, sync=False,
                               reason="defer transpose")
            o_acc2 = oap.tile([128, 128], BF16, tag="oacc")
            nc.vector.scalar_tensor_tensor(
                o_acc2, tp, s_edup[:, t:t + 1], o_acc, MUL, ADD)
            o_acc = o_acc2

        # ================= ENCODER =================
        for t in range(t_enc):
            xp_t = xpool.tile([66, BL], BF16, tag="xp")
            nc.sync.dma_start(xp_t, xp_d[t, :, :])
            psA = gpool.tile([128, 256], f, tag="gA")
            psB = gbc.tile([128, 128], f, tag="gB")
            psC = gbc.tile([128, 128], f, tag="gC")
            last_mm = gate_mms(psA, psB, psC, s_whhenc, 0,
                               xlhs=s_wxenc, xrhs=xp_t)
            flush_oacc(after=last_mm)
            if debug and t == 0:
                gcp = work.tile([128, 512], f, tag="gdbg")
                nc.vector.tensor_copy(gcp[:, 0:256], psA)
                nc.vector.tensor_copy(gcp[:, 384:512], psB)
                nc.vector.tensor_copy(gcp[:, 256:384], psC)
                nc.sync.dma_start(gdbg_d[:, :], gcp)
            cell(psA, psB, psC, t)

        if debug:
            nc.sync.dma_start(cdbg_d[:, :, :], c_prev)
            nc.sync.dma_start(hdbg_d[:, 0, :], hT_prev[0])
            nc.sync.dma_start(hdbg_d[:, 1, :], hT_prev[1])
            nc.sync.dma_start(odbg_d[:, :], o_acc)

        flush_oacc()

        # ===== boundary: hT_enc = o_sel^T * tanh(c_final) =====
        tce = work.tile([128, 2, BL], BF16, tag="tct")
        nc.scalar.activation(tce, c_prev, Tanh)
        tpe = tpp.tile([128, 128], BF16, tag="tp")
        nc.tensor.transpose(tpe, o_acc, s_identb)
        o_selT = work.tile([128, 128], BF16, tag="osel")
        nc.vector.tensor_copy(o_selT, tpe)
        hT_b = (hpool.tile([128, BL], BF16, tag="hT0", name="hTb0"),
                hpool.tile([128, BL], BF16, tag="hT1", name="hTb1"))
        for k in (0, 1):
            nc.vector.tensor_mul(hT_b[k], o_selT[:, 64 * k:64 * k + 64],
                                 tce[:, k, :])
        hT_prev = hT_b
        if debug:
            nc.sync.dma_start(hbdbg_d[:, 0, :], hT_b[0])
            nc.sync.dma_start(hbdbg_d[:, 1, :], hT_b[1])

        # ================= DECODER =================
        # y = out_W @ h + out_b computed in-loop: 2 small matmuls per step
        # accumulate into a persistent PSUM group bank (8 steps per bank,
        # opened by a rank-1 out_b matmul); one ACT copy + one DMA per group.
        psy = None
        for j in range(t_dec):
            psA = gpool.tile([128, 256], f, tag="gA")
            psB = gbc.tile([128, 128], f, tag="gB")
            psC = gbc.tile([128, 128], f, tag="gC")
            if j == 0:
                last_mm = gate_mms(psA, psB, psC, s_whhdec, 1,
                                   xlhs=s_wxdec, xrhs=s_x0p)
            else:
                last_mm = gate_mms(psA, psB, psC, s_wcomb, 2)
            cell(psA, psB, psC, None)
            g8 = j % 8
            ymms = [last_mm]
            if g8 == 0:
                psy = tpp.tile([D, 512], f, tag="tp", name=f"psy{j}")
                ymms.append(nc.tensor.matmul(psy, s_obT, s_onesy,
                                             start=True, stop=False,
                                             skip_group_check=True))
            for k in (0, 1):
                ymms.append(nc.tensor.matmul(
                    psy[:, 64 * g8:64 * g8 + 64], s_outwT[:, k, :],
                    hT_prev[k], start=False, stop=(k == 1),
                    skip_group_check=True))
            chain(ymms)
            if g8 == 7 or j == t_dec - 1:
                cnt = g8 + 1
                y_sb = work.tile([D, 512], f, tag="ysb")
                nc.scalar.copy(y_sb[:, 0:64 * cnt], psy[:, 0:64 * cnt])
                nc.sync.dma_start(yt_d[:, j - g8 + 1:j + 2, :],
                                  y_sb[:, 0:64 * cnt])

    nc.compile()
    return nc


def _prep_host(inputs, t_enc=T, t_dec=TDEC):
    """Build per-core in_maps from full inputs (numpy)."""
    x = np.asarray(inputs["input_tensor"], np.float32)
    tgt = np.asarray(inputs["target_tensor"], np.float32)
    lens = np.asarray(inputs["lens"]).astype(np.int64)

    eWih = np.asarray(inputs["enc_Wih"], np.float32)
    eWhh = np.asarray(inputs["enc_Whh"], np.float32)
    eb = (np.asarray(inputs["enc_bih"], np.float32)
          + np.asarray(inputs["enc_bhh"], np.float32))
    dWih = np.asarray(inputs["dec_Wih"], np.float32)
    dWhh = np.asarray(inputs["dec_Whh"], np.float32)
    db = (np.asarray(inputs["dec_bih"], np.float32)
          + np.asarray(inputs["dec_bhh"], np.float32))
    oW = np.asarray(inputs["out_W"], np.float32)
    ob = np.asarray(inputs["out_b"], np.float32)

    wcomb_full = dWhh + dWih @ oW          # [G4, H]
    bcomb = db + dWih @ ob                 # [G4]

    def chunked_x(W, freeze_big):
        # -> [66, 8, 128]: rows 0:64 x-weights^T, row 64 unused, row 65 freeze
        out = np.zeros((66, 8, 128), np.float32)
        for m, (r0, r1) in enumerate(CHUNK_ROWS):
            out[0:64, m, :] = W[r0:r1, :].T
            if freeze_big and m in (0, 1):
                out[65, m, :] = -BIG
            elif freeze_big and m in (2, 3):
                out[65, m, :] = BIG
        return out.astype(BF)

    def chunked_b(b):
        return np.stack([b[r0:r1] for (r0, r1) in CHUNK_ROWS])

    def chunked_h(W):
        # -> [128, 2, 8, 128]
        out = np.zeros((128, 2, 8, 128), np.float32)
        for m, (r0, r1) in enumerate(CHUNK_ROWS):
            for k in (0, 1):
                out[:, k, m, :] = W[r0:r1, 128 * k:128 * (k + 1)].T
        return out.astype(BF)

    wxenc = chunked_x(eWih, True)
    wxdec = chunked_x(dWih, False)
    whhenc = chunked_h(eWhh)
    whhdec = chunked_h(dWhh)
    wcomb = chunked_h(wcomb_full)
    # bias sets: 0=enc, 1=dec step0, 2=comb. Banks: A=m0-3, B=m6-7, C=m4-5.
    bsets = [chunked_b(eb), chunked_b(db), chunked_b(bcomb)]  # each [8, 128]
    biasA = np.stack([bs[0:4] for bs in bsets], 1).astype(BF)  # [4, 3, 128]
    biasB = np.stack([bs[6:8] for bs in bsets], 1).astype(BF)  # [2, 3, 128]
    biasC = np.stack([bs[4:6] for bs in bsets], 1).astype(BF)  # [2, 3, 128]
    obT = ob[None, :].astype(BF)
    onesy = np.ones((1, 512), np.float32).astype(BF)
    # blockones: rows 0-3 x cols 0-255 = 4x64 block-diag (bank A opener);
    # rows 4-5 x cols 256-383 = 2x64 block-diag (bank B/C opener)
    blockones = np.zeros((8, 512), np.float32)
    for m in range(4):
        blockones[m, 64 * m:64 * m + 64] = 1.0
    for m in range(2):
        blockones[4 + m, 256 + 64 * m:256 + 64 * m + 64] = 1.0
    blockones = blockones.astype(BF)
    ident = np.eye(128, dtype=np.float32).astype(BF)
    outwT = oW.T.reshape(2, 128, D).transpose(1, 0, 2).astype(BF).copy()
    outb = ob[:, None].astype(np.float32).copy()

    tt = np.arange(t_enc)[None, :]
    in_maps = []
    for c in range(NCORES):
        b0 = c * BL
        xs = x[b0:b0 + BL, :t_enc, :]                # [BL,t,D]
        xp = np.empty((t_enc, 66, BL), np.float32)
        xp[:, 0:D, :] = xs.transpose(1, 2, 0)
        xp[:, D, :] = 1.0
        lc = lens[b0:b0 + BL]
        mbar = (tt >= lc[:, None]).astype(np.float32)   # [BL,t]
        xp[:, D + 1, :] = mbar.T
        efreeze = (tt == (lc[:, None] - 1)).astype(np.float32)  # [BL,t]
        edup = np.concatenate([efreeze, efreeze], 0)    # [128,t]
        x0p = np.zeros((66, BL), np.float32)
        x0p[0:D, :] = tgt[b0:b0 + BL, 0, :].T
        x0p[D, :] = 1.0
        in_maps.append({
            "xp": np.ascontiguousarray(xp).astype(BF),
            "x0p": x0p.astype(BF),
            "wxenc": wxenc, "wxdec": wxdec,
            "whhenc": whhenc, "whhdec": whhdec, "wcomb": wcomb,
            "biasA": biasA, "biasB": biasB, "biasC": biasC,
            "obT": obT, "onesy": onesy,
            "blockones": blockones, "ident": ident,
            "edup": np.ascontiguousarray(edup),
            "outwT": outwT, "outb": outb,
        })
    return in_maps, lens


def kernel(**inputs) -> np.ndarray:
    global _PROGRAM, LAST_RESULTS
    if _PROGRAM is None:
        _PROGRAM = build_program()
    nc = _PROGRAM
    in_maps, lens = _prep_host(inputs)
    res = run_bass_kernel_spmd(nc, in_maps, core_ids=list(range(NCORES)))
    LAST_RESULTS = res
    out = np.zeros((B, T, D), np.float32)
    for c in range(NCORES):
        yt = res.results[c]["yt"]                      # [D, T, BL]
        out[c * BL:(c + 1) * BL] = yt.transpose(2, 1, 0)
    mask = (np.arange(T)[None, :] < lens[:, None])[:, :, None]
    out *= mask
    out[:, 0, :] = 0.0
    return out


# revision 46
# speedup vs baseline: 1.1998x; 1.0716x over previous
"""Trainium2 Bass kernel for the LSTM seq2seq autoencoder (layout B).

Strategy:
  - Data-parallel over batch: B=512 -> 64 rows per core on 8 cores.
  - Gates-on-partitions layout: gate preactivations live in one PSUM bank
    [128, 512] = 8 chunks x 64 batch cols, chunk order [i0 i1 f0 f1 o0 o1 g0 g1].
    Each step: 16 (h) + 8 (x, encoder) LDW+MM pairs of N=64 (FWL-eligible
    bf16 weights, LDWEIGHTS hidden behind matmuls) + one rank-8 bias matmul
    (decoder) whose rhs is a block-indicator [8, 512].
  - h^T [128, 2, 64] is produced directly by the DVE h-mul (no per-step
    transposes or PSUM->SBUF copies) and is the rhs of the next step's MMs.
  - Encoder length masking: c frozen by forcing i -> -BIG, f -> +BIG via the
    mbar row of xp; o captured at the freeze step via PE transpose +
    one fused scalar_tensor_tensor (o_acc = o^T_t * e_t + o_acc).
  - Decoder feedback folded: W_comb = Whh + Wih_dec @ out_W.
  - y = out_W @ h + out_b deferred entirely to a batched end-phase GEMM over
    h^T tiles dumped to DRAM each step (DMA engines are otherwise idle).
"""

import numpy as np
import ml_dtypes
from contextlib import ExitStack

import concourse.bass as bass
import concourse.bacc as bacc
import concourse.mybir as mybir
import concourse.tile as tile
from concourse.tile import add_dep_helper
from concourse.bass_utils import run_bass_kernel_spmd

B, T, D, H = 512, 512, 64, 256
G4 = 4 * H  # 1024
NCORES = 8
BL = B // NCORES  # 64
TDEC = T - 1      # 511 decoder steps
BIG = 30000.0
F32 = mybir.dt.float32
BF16 = mybir.dt.bfloat16
BF = ml_dtypes.bfloat16

_PROGRAM = None
LAST_RESULTS = None

# chunk order on the 512 free cols: [i0 i1 f0 f1 o0 o1 g0 g1]
# torch gate rows: i=[0,256) f=[256,512) g=[512,768) o=[768,1024)
CHUNK_ROWS = [(0, 128), (128, 256), (256, 384), (384, 512),
              (768, 896), (896, 1024), (512, 640), (640, 768)]

Sig = mybir.ActivationFunctionType.Sigmoid
Tanh = mybir.ActivationFunctionType.Tanh
Ident = mybir.ActivationFunctionType.Identity
MUL = mybir.AluOpType.mult
ADD = mybir.AluOpType.add


def build_program(t_enc=T, t_dec=TDEC, debug=False):
    nc = bacc.Bacc(None, target_bir_lowering=False)
    f = F32
    if debug:
        gdbg_d = nc.dram_tensor("gdbg", [128, 512], F32, kind="ExternalOutput")
        cdbg_d = nc.dram_tensor("cdbg", [128, 2, BL], F32, kind="ExternalOutput")
        hdbg_d = nc.dram_tensor("hdbg", [128, 2, BL], BF16, kind="ExternalOutput")
        odbg_d = nc.dram_tensor("odbg", [128, 128], BF16, kind="ExternalOutput")
        hbdbg_d = nc.dram_tensor("hbdbg", [128, 2, BL], BF16, kind="ExternalOutput")
    xp_d = nc.dram_tensor("xp", [t_enc, 66, BL], BF16, kind="ExternalInput")
    x0p_d = nc.dram_tensor("x0p", [66, BL], BF16, kind="ExternalInput")
    wxenc_d = nc.dram_tensor("wxenc", [66, 8, 128], BF16, kind="ExternalInput")
    wxdec_d = nc.dram_tensor("wxdec", [66, 8, 128], BF16, kind="ExternalInput")
    whhenc_d = nc.dram_tensor("whhenc", [128, 2, 8, 128], BF16, kind="ExternalInput")
    whhdec_d = nc.dram_tensor("whhdec", [128, 2, 8, 128], BF16, kind="ExternalInput")
    wcomb_d = nc.dram_tensor("wcomb", [128, 2, 8, 128], BF16, kind="ExternalInput")
    biasA_d = nc.dram_tensor("biasA", [4, 3, 128], BF16, kind="ExternalInput")
    biasB_d = nc.dram_tensor("biasB", [2, 3, 128], BF16, kind="ExternalInput")
    biasC_d = nc.dram_tensor("biasC", [2, 3, 128], BF16, kind="ExternalInput")
    obT_d = nc.dram_tensor("obT", [1, D], BF16, kind="ExternalInput")
    onesy_d = nc.dram_tensor("onesy", [1, 512], BF16, kind="ExternalInput")
    blockones_d = nc.dram_tensor("blockones", [8, 512], BF16, kind="ExternalInput")
    ident_d = nc.dram_tensor("ident", [128, 128], BF16, kind="ExternalInput")
    edup_d = nc.dram_tensor("edup", [128, t_enc], F32, kind="ExternalInput")
    outwT_d = nc.dram_tensor("outwT", [128, 2, D], BF16, kind="ExternalInput")
    outb_d = nc.dram_tensor("outb", [D, 1], F32, kind="ExternalInput")
    yt_d = nc.dram_tensor("yt", [D, t_dec + 1, BL], F32, kind="ExternalOutput")

    with ExitStack() as ctx:
        tc = ctx.enter_context(tile.TileContext(nc))
        singles = ctx.enter_context(tc.tile_pool(name="singles", bufs=1))
        xpool = ctx.enter_context(tc.tile_pool(name="xpool", bufs=6))
        work = ctx.enter_context(tc.tile_pool(name="work", bufs=3))
        hpool = ctx.enter_context(tc.tile_pool(name="hpool", bufs=2))
        cpool = ctx.enter_context(tc.tile_pool(name="cpool", bufs=2))
        oap = ctx.enter_context(tc.tile_pool(name="oap", bufs=2))
        gpool = ctx.enter_context(
            tc.tile_pool(name="gpool", bufs=3, space=bass.MemorySpace.PSUM))
        gbc = ctx.enter_context(
            tc.tile_pool(name="gbc", bufs=2, space=bass.MemorySpace.PSUM))
        tpp = ctx.enter_context(
            tc.tile_pool(name="tpp", bufs=1, space=bass.MemorySpace.PSUM))

        # ---- persistent constants ----
        s_wxenc = singles.tile([66, 8, 128], BF16)
        nc.sync.dma_start(s_wxenc, wxenc_d[:, :, :])
        s_wxdec = singles.tile([66, 8, 128], BF16)
        nc.sync.dma_start(s_wxdec, wxdec_d[:, :, :])
        s_whhenc = singles.tile([128, 2, 8, 128], BF16)
        nc.sync.dma_start(s_whhenc, whhenc_d[:, :, :, :])
        s_whhdec = singles.tile([128, 2, 8, 128], BF16)
        nc.sync.dma_start(s_whhdec, whhdec_d[:, :, :, :])
        s_wcomb = singles.tile([128, 2, 8, 128], BF16)
        nc.sync.dma_start(s_wcomb, wcomb_d[:, :, :, :])
        s_biasA = singles.tile([4, 3, 128], BF16)
        nc.sync.dma_start(s_biasA, biasA_d[:, :, :])
        s_biasB = singles.tile([2, 3, 128], BF16)
        nc.sync.dma_start(s_biasB, biasB_d[:, :, :])
        s_biasC = singles.tile([2, 3, 128], BF16)
        nc.sync.dma_start(s_biasC, biasC_d[:, :, :])
        s_obT = singles.tile([1, D], BF16)
        nc.sync.dma_start(s_obT, obT_d[:, :])
        s_onesy = singles.tile([1, 512], BF16)
        nc.sync.dma_start(s_onesy, onesy_d[:, :])
        s_bonesA = singles.tile([4, 256], BF16)
        nc.sync.dma_start(s_bonesA, blockones_d[0:4, 0:256])
        s_bonesBC = singles.tile([2, 128], BF16)
        nc.sync.dma_start(s_bonesBC, blockones_d[4:6, 256:384])
        s_identb = singles.tile([128, 128], BF16)
        nc.sync.dma_start(s_identb, ident_d[:, :])
        s_edup = singles.tile([128, t_enc], F32)
        nc.sync.dma_start(s_edup, edup_d[:, :])
        s_outwT = singles.tile([128, 2, D], BF16)
        nc.sync.dma_start(s_outwT, outwT_d[:, :, :])
        s_outb = singles.tile([D, 1], f)
        nc.sync.dma_start(s_outb, outb_d[:, :])
        s_x0p = singles.tile([66, BL], BF16)
        nc.sync.dma_start(s_x0p, x0p_d[:, :])

        # ---- initial state ----
        c_prev = singles.tile([128, 2, BL], f, tag="c0")
        nc.vector.memset(c_prev, 0.0)
        hT_i0 = singles.tile([128, BL], BF16, tag="hi0")
        nc.vector.memset(hT_i0, 0.0)
        hT_i1 = singles.tile([128, BL], BF16, tag="hi1")
        nc.vector.memset(hT_i1, 0.0)
        hT_prev = (hT_i0, hT_i1)
        o_acc = singles.tile([128, 128], BF16, tag="oacc0")
        nc.vector.memset(o_acc, 0.0)

        def chain(insts):
            for a, b in zip(insts[1:], insts[:-1]):
                add_dep_helper(a.ins, b.ins, sync=False, reason="pe-order")

        # gate chunk m -> (bank, col offset): A=i,f (m0-3), B=g (m6,7), C=o (m4,5)
        def bank_slice(psA, psB, psC, m):
            if m < 4:
                return psA[:, 64 * m:64 * m + 64]
            if m >= 6:
                return psB[:, 64 * (m - 6):64 * (m - 6) + 64]
            return psC[:, 64 * (m - 4):64 * (m - 4) + 64]

        def gate_mms(psA, psB, psC, whh, bset, xlhs=None, xrhs=None):
            """All matmuls of one step. Gates split across three PSUM banks
            so each ACT read waits only on its own bank's writers (PSUM
            bank-level write/read serialization is a hardware constraint).
            One start=True (bias) matmul opens each bank; bias + x MMs run in
            the PE-idle window; h MMs go bank-A-first. The explicit chain
            pins the scheduler to this PE order."""
            mms = [
                nc.tensor.matmul(psA, s_biasA[:, bset, :], s_bonesA,
                                 start=True, stop=False, skip_group_check=True),
                nc.tensor.matmul(psB, s_biasB[:, bset, :], s_bonesBC,
                                 start=True, stop=False, skip_group_check=True),
                nc.tensor.matmul(psC, s_biasC[:, bset, :], s_bonesBC,
                                 start=True, stop=False, skip_group_check=True),
            ]
            if xlhs is not None:
                for m in (0, 1, 2, 3, 6, 7, 4, 5):
                    mms.append(nc.tensor.matmul(bank_slice(psA, psB, psC, m),
                                                xlhs[:, m, :], xrhs,
                                                start=False, stop=False,
                                                skip_group_check=True))
            # k0 group first (gated only by h0), then k1 group (h1); within
            # each group bank A first so sig_if's bank completes earliest
            for k in (0, 1):
                for m in (0, 1, 2, 3, 6, 7, 4, 5):
                    mms.append(nc.tensor.matmul(
                        bank_slice(psA, psB, psC, m),
                        whh[:, k, m, :], hT_prev[k],
                        start=False, stop=(k == 1),
                        skip_group_check=True))
            chain(mms)
            return mms[-1]

        def cell(psA, psB, psC, enc_t):
            """LSTM cell elementwise phase. Updates c_prev/hT_prev (+o_acc)."""
            nonlocal c_prev, hT_prev, o_acc
            if_t = work.tile([128, 256], BF16, tag="ift")
            nc.scalar.activation(if_t, psA, Sig)
            g_t = work.tile([128, 128], BF16, tag="gt")
            nc.scalar.activation(g_t, psB, Tanh)
            o_t = work.tile([128, 128], BF16, tag="ot")
            nc.scalar.activation(o_t, psC, Sig)
            c_new = cpool.tile([128, 2, BL], f, tag="c")
            tct = work.tile([128, 2, BL], BF16, tag="tct")
            hT_new = (hpool.tile([128, BL], BF16, tag="hT0", name="hT0"),
                      hpool.tile([128, BL], BF16, tag="hT1", name="hT1"))
            for k in (0, 1):
                sl = slice(64 * k, 64 * k + 64)
                fc = work.tile([128, BL], f, tag=f"fc{k}")
                nc.vector.tensor_mul(fc, if_t[:, 128 + 64 * k:192 + 64 * k],
                                     c_prev[:, k, :])
                ig = work.tile([128, BL], f, tag=f"ig{k}")
                nc.vector.tensor_mul(ig, if_t[:, sl], g_t[:, sl])
                nc.vector.tensor_add(c_new[:, k, :], fc, ig)
                nc.scalar.activation(tct[:, k, :], c_new[:, k, :], Tanh)
                nc.vector.tensor_mul(hT_new[k], o_t[:, sl], tct[:, k, :])
            if enc_t is not None:
                pending_o[0] = (o_t, enc_t)
            c_prev = c_new
            hT_prev = hT_new

        pending_o = [None]

        def flush_oacc(after=None):
            """Deferred o_acc capture: the PE transpose of step t's o is
            pinned after step t+1's matmuls so it never blocks the PE FIFO
            while waiting on sig_o."""
            nonlocal o_acc
            if pending_o[0] is None:
                return
            o_t, t = pending_o[0]
            pending_o[0] = None
            tp = tpp.tile([128, 128], BF16, tag="tp")
            tri = nc.tensor.transpose(tp, o_t, s_identb)
            if after is not None:
                add_dep_helper(tri.ins, after.ins, sync=False,
                               reason="defer transpose")
            o_acc2 = oap.tile([128, 128], BF16, tag="oacc")
            nc.vector.scalar_tensor_tensor(
                o_acc2, tp, s_edup[:, t:t + 1], o_acc, MUL, ADD)
            o_acc = o_acc2

        # ================= ENCODER =================
        for t in range(t_enc):
            xp_t = xpool.tile([66, BL], BF16, tag="xp")
            nc.sync.dma_start(xp_t, xp_d[t, :, :])
            psA = gpool.tile([128, 256], f, tag="gA")
            psB = gbc.tile([128, 128], f, tag="gB")
            psC = gbc.tile([128, 128], f, tag="gC")
            last_mm = gate_mms(psA, psB, psC, s_whhenc, 0,
                               xlhs=s_wxenc, xrhs=xp_t)
            flush_oacc(after=last_mm)
            if debug and t == 0:
                gcp = work.tile([128, 512], f, tag="gdbg")
                nc.vector.tensor_copy(gcp[:, 0:256], psA)
                nc.vector.tensor_copy(gcp[:, 384:512], psB)
                nc.vector.tensor_copy(gcp[:, 256:384], psC)
                nc.sync.dma_start(gdbg_d[:, :], gcp)
            cell(psA, psB, psC, t)

        if debug:
            nc.sync.dma_start(cdbg_d[:, :, :], c_prev)
            nc.sync.dma_start(hdbg_d[:, 0, :], hT_prev[0])
            nc.sync.dma_start(hdbg_d[:, 1, :], hT_prev[1])
            nc.sync.dma_start(odbg_d[:, :], o_acc)

        flush_oacc()

        # ===== boundary: hT_enc = o_sel^T * tanh(c_final) =====
        tce = work.tile([128, 2, BL], BF16, tag="tct")
        nc.scalar.activation(tce, c_prev, Tanh)
        tpe = tpp.tile([128, 128], BF16, tag="tp")
        nc.tensor.transpose(tpe, o_acc, s_identb)
        o_selT = work.tile([128, 128], BF16, tag="osel")
        nc.vector.tensor_copy(o_selT, tpe)
        hT_b = (hpool.tile([128, BL], BF16, tag="hT0", name="hTb0"),
                hpool.tile([128, BL], BF16, tag="hT1", name="hTb1"))
        for k in (0, 1):
            nc.vector.tensor_mul(hT_b[k], o_selT[:, 64 * k:64 * k + 64],
                                 tce[:, k, :])
        hT_prev = hT_b
        if debug:
            nc.sync.dma_start(hbdbg_d[:, 0, :], hT_b[0])
            nc.sync.dma_start(hbdbg_d[:, 1, :], hT_b[1])

        # ================= DECODER =================
        # y = out_W @ h + out_b computed in-loop: 2 small matmuls per step
        # accumulate into a persistent PSUM group bank (8 steps per bank,
        # opened by a rank-1 out_b matmul); one ACT copy + one DMA per group.
        psy_box = [None]
        pending_y = [None]

        def flush_y(after=None):
            """Deferred y matmuls: step j's out_W@h_j runs behind step j+1's
            gate matmuls in the PE FIFO so it never delays the next burst."""
            if pending_y[0] is None:
                return
            j, hT = pending_y[0]
            pending_y[0] = None
            g8 = j % 8
            ymms = [] if after is None else [after]
            if g8 == 0:
                psy_box[0] = tpp.tile([D, 512], f, tag="tp", name=f"psy{j}")
                ymms.append(nc.tensor.matmul(psy_box[0], s_obT, s_onesy,
                                             start=True, stop=False,
                                             skip_group_check=True))
            psy = psy_box[0]
            for k in (0, 1):
                ymms.append(nc.tensor.matmul(
                    psy[:, 64 * g8:64 * g8 + 64], s_outwT[:, k, :],
                    hT[k], start=False, stop=(k == 1),
                    skip_group_check=True))
            chain(ymms)
            if g8 == 7 or j == t_dec - 1:
                cnt = g8 + 1
                y_sb = work.tile([D, 512], f, tag="ysb")
                nc.scalar.copy(y_sb[:, 0:64 * cnt], psy[:, 0:64 * cnt])
                nc.sync.dma_start(yt_d[:, j - g8 + 1:j + 2, :],
                                  y_sb[:, 0:64 * cnt])

        for j in range(t_dec):
            psA = gpool.tile([128, 256], f, tag="gA")
            psB = gbc.tile([128, 128], f, tag="gB")
            psC = gbc.tile([128, 128], f, tag="gC")
            if j == 0:
                last_mm = gate_mms(psA, psB, psC, s_whhdec, 1,
                                   xlhs=s_wxdec, xrhs=s_x0p)
            else:
                last_mm = gate_mms(psA, psB, psC, s_wcomb, 2)
            flush_y(after=last_mm)
            cell(psA, psB, psC, None)
            pending_y[0] = (j, hT_prev)
        flush_y()

    nc.compile()
    return nc


def _prep_host(inputs, t_enc=T, t_dec=TDEC):
    """Build per-core in_maps from full inputs (numpy)."""
    x = np.asarray(inputs["input_tensor"], np.float32)
    tgt = np.asarray(inputs["target_tensor"], np.float32)
    lens = np.asarray(inputs["lens"]).astype(np.int64)

    eWih = np.asarray(inputs["enc_Wih"], np.float32)
    eWhh = np.asarray(inputs["enc_Whh"], np.float32)
    eb = (np.asarray(inputs["enc_bih"], np.float32)
          + np.asarray(inputs["enc_bhh"], np.float32))
    dWih = np.asarray(inputs["dec_Wih"], np.float32)
    dWhh = np.asarray(inputs["dec_Whh"], np.float32)
    db = (np.asarray(inputs["dec_bih"], np.float32)
          + np.asarray(inputs["dec_bhh"], np.float32))
    oW = np.asarray(inputs["out_W"], np.float32)
    ob = np.asarray(inputs["out_b"], np.float32)

    wcomb_full = dWhh + dWih @ oW          # [G4, H]
    bcomb = db + dWih @ ob                 # [G4]

    def chunked_x(W, freeze_big):
        # -> [66, 8, 128]: rows 0:64 x-weights^T, row 64 unused, row 65 freeze
        out = np.zeros((66, 8, 128), np.float32)
        for m, (r0, r1) in enumerate(CHUNK_ROWS):
            out[0:64, m, :] = W[r0:r1, :].T
            if freeze_big and m in (0, 1):
                out[65, m, :] = -BIG
            elif freeze_big and m in (2, 3):
                out[65, m, :] = BIG
        return out.astype(BF)

    def chunked_b(b):
        return np.stack([b[r0:r1] for (r0, r1) in CHUNK_ROWS])

    def chunked_h(W):
        # -> [128, 2, 8, 128]
        out = np.zeros((128, 2, 8, 128), np.float32)
        for m, (r0, r1) in enumerate(CHUNK_ROWS):
            for k in (0, 1):
                out[:, k, m, :] = W[r0:r1, 128 * k:128 * (k + 1)].T
        return out.astype(BF)

    wxenc = chunked_x(eWih, True)
    wxdec = chunked_x(dWih, False)
    whhenc = chunked_h(eWhh)
    whhdec = chunked_h(dWhh)
    wcomb = chunked_h(wcomb_full)
    # bias sets: 0=enc, 1=dec step0, 2=comb. Banks: A=m0-3, B=m6-7, C=m4-5.
    bsets = [chunked_b(eb), chunked_b(db), chunked_b(bcomb)]  # each [8, 128]
    biasA = np.stack([bs[0:4] for bs in bsets], 1).astype(BF)  # [4, 3, 128]
    biasB = np.stack([bs[6:8] for bs in bsets], 1).astype(BF)  # [2, 3, 128]
    biasC = np.stack([bs[4:6] for bs in bsets], 1).astype(BF)  # [2, 3, 128]
    obT = ob[None, :].astype(BF)
    onesy = np.ones((1, 512), np.float32).astype(BF)
    # blockones: rows 0-3 x cols 0-255 = 4x64 block-diag (bank A opener);
    # rows 4-5 x cols 256-383 = 2x64 block-diag (bank B/C opener)
    blockones = np.zeros((8, 512), np.float32)
    for m in range(4):
        blockones[m, 64 * m:64 * m + 64] = 1.0
    for m in range(2):
        blockones[4 + m, 256 + 64 * m:256 + 64 * m + 64] = 1.0
    blockones = blockones.astype(BF)
    ident = np.eye(128, dtype=np.float32).astype(BF)
    outwT = oW.T.reshape(2, 128, D).transpose(1, 0, 2).astype(BF).copy()
    outb = ob[:, None].astype(np.float32).copy()

    tt = np.arange(t_enc)[None, :]
    in_maps = []
    for c in range(NCORES):
        b0 = c * BL
        xs = x[b0:b0 + BL, :t_enc, :]                # [BL,t,D]
        xp = np.empty((t_enc, 66, BL), np.float32)
        xp[:, 0:D, :] = xs.transpose(1, 2, 0)
        xp[:, D, :] = 1.0
        lc = lens[b0:b0 + BL]
        mbar = (tt >= lc[:, None]).astype(np.float32)   # [BL,t]
        xp[:, D + 1, :] = mbar.T
        efreeze = (tt == (lc[:, None] - 1)).astype(np.float32)  # [BL,t]
        edup = np.concatenate([efreeze, efreeze], 0)    # [128,t]
        x0p = np.zeros((66, BL), np.float32)
        x0p[0:D, :] = tgt[b0:b0 + BL, 0, :].T
        x0p[D, :] = 1.0
        in_maps.append({
            "xp": np.ascontiguousarray(xp).astype(BF),
            "x0p": x0p.astype(BF),
            "wxenc": wxenc, "wxdec": wxdec,
            "whhenc": whhenc, "whhdec": whhdec, "wcomb": wcomb,
            "biasA": biasA, "biasB": biasB, "biasC": biasC,
            "obT": obT, "onesy": onesy,
            "blockones": blockones, "ident": ident,
            "edup": np.ascontiguousarray(edup),
            "outwT": outwT, "outb": outb,
        })
    return in_maps, lens


def kernel(**inputs) -> np.ndarray:
    global _PROGRAM, LAST_RESULTS
    if _PROGRAM is None:
        _PROGRAM = build_program()
    nc = _PROGRAM
    in_maps, lens = _prep_host(inputs)
    res = run_bass_kernel_spmd(nc, in_maps, core_ids=list(range(NCORES)))
    LAST_RESULTS = res
    out = np.zeros((B, T, D), np.float32)
    for c in range(NCORES):
        yt = res.results[c]["yt"]                      # [D, T, BL]
        out[c * BL:(c + 1) * BL] = yt.transpose(2, 1, 0)
    mask = (np.arange(T)[None, :] < lens[:, None])[:, :, None]
    out *= mask
    out[:, 0, :] = 0.0
    return out


# revision 47
# speedup vs baseline: 1.2290x; 1.0244x over previous
"""Trainium2 Bass kernel for the LSTM seq2seq autoencoder (layout B).

Strategy:
  - Data-parallel over batch: B=512 -> 64 rows per core on 8 cores.
  - Gates-on-partitions layout: gate preactivations live in one PSUM bank
    [128, 512] = 8 chunks x 64 batch cols, chunk order [i0 i1 f0 f1 o0 o1 g0 g1].
    Each step: 16 (h) + 8 (x, encoder) LDW+MM pairs of N=64 (FWL-eligible
    bf16 weights, LDWEIGHTS hidden behind matmuls) + one rank-8 bias matmul
    (decoder) whose rhs is a block-indicator [8, 512].
  - h^T [128, 2, 64] is produced directly by the DVE h-mul (no per-step
    transposes or PSUM->SBUF copies) and is the rhs of the next step's MMs.
  - Encoder length masking: c frozen by forcing i -> -BIG, f -> +BIG via the
    mbar row of xp; o captured at the freeze step via PE transpose +
    one fused scalar_tensor_tensor (o_acc = o^T_t * e_t + o_acc).
  - Decoder feedback folded: W_comb = Whh + Wih_dec @ out_W.
  - y = out_W @ h + out_b deferred entirely to a batched end-phase GEMM over
    h^T tiles dumped to DRAM each step (DMA engines are otherwise idle).
"""

import numpy as np
import ml_dtypes
from contextlib import ExitStack

import concourse.bass as bass
import concourse.bacc as bacc
import concourse.mybir as mybir
import concourse.tile as tile
from concourse.tile import add_dep_helper
from concourse.bass_utils import run_bass_kernel_spmd

B, T, D, H = 512, 512, 64, 256
G4 = 4 * H  # 1024
NCORES = 8
BL = B // NCORES  # 64
TDEC = T - 1      # 511 decoder steps
BIG = 30000.0
F32 = mybir.dt.float32
BF16 = mybir.dt.bfloat16
BF = ml_dtypes.bfloat16

_PROGRAM = None
LAST_RESULTS = None

# chunk order on the 512 free cols: [i0 i1 f0 f1 o0 o1 g0 g1]
# torch gate rows: i=[0,256) f=[256,512) g=[512,768) o=[768,1024)
CHUNK_ROWS = [(0, 128), (128, 256), (256, 384), (384, 512),
              (768, 896), (896, 1024), (512, 640), (640, 768)]

Sig = mybir.ActivationFunctionType.Sigmoid
Tanh = mybir.ActivationFunctionType.Tanh
Ident = mybir.ActivationFunctionType.Identity
MUL = mybir.AluOpType.mult
ADD = mybir.AluOpType.add


def build_program(t_enc=T, t_dec=TDEC, debug=False):
    nc = bacc.Bacc(None, target_bir_lowering=False)
    f = F32
    if debug:
        gdbg_d = nc.dram_tensor("gdbg", [128, 512], F32, kind="ExternalOutput")
        cdbg_d = nc.dram_tensor("cdbg", [128, 2, BL], F32, kind="ExternalOutput")
        hdbg_d = nc.dram_tensor("hdbg", [128, 2, BL], BF16, kind="ExternalOutput")
        odbg_d = nc.dram_tensor("odbg", [128, 128], BF16, kind="ExternalOutput")
        hbdbg_d = nc.dram_tensor("hbdbg", [128, 2, BL], BF16, kind="ExternalOutput")
    xp_d = nc.dram_tensor("xp", [t_enc, 66, BL], BF16, kind="ExternalInput")
    x0p_d = nc.dram_tensor("x0p", [66, BL], BF16, kind="ExternalInput")
    wxenc_d = nc.dram_tensor("wxenc", [66, 8, 128], BF16, kind="ExternalInput")
    wxdec_d = nc.dram_tensor("wxdec", [66, 8, 128], BF16, kind="ExternalInput")
    whhenc_d = nc.dram_tensor("whhenc", [128, 2, 8, 128], BF16, kind="ExternalInput")
    whhdec_d = nc.dram_tensor("whhdec", [128, 2, 8, 128], BF16, kind="ExternalInput")
    wcomb_d = nc.dram_tensor("wcomb", [128, 2, 8, 128], BF16, kind="ExternalInput")
    biasA_d = nc.dram_tensor("biasA", [4, 3, 128], BF16, kind="ExternalInput")
    biasB_d = nc.dram_tensor("biasB", [2, 3, 128], BF16, kind="ExternalInput")
    biasC_d = nc.dram_tensor("biasC", [2, 3, 128], BF16, kind="ExternalInput")
    obT_d = nc.dram_tensor("obT", [1, D], BF16, kind="ExternalInput")
    onesy_d = nc.dram_tensor("onesy", [1, 512], BF16, kind="ExternalInput")
    blockones_d = nc.dram_tensor("blockones", [8, 512], BF16, kind="ExternalInput")
    ident_d = nc.dram_tensor("ident", [128, 128], BF16, kind="ExternalInput")
    edup_d = nc.dram_tensor("edup", [128, t_enc], F32, kind="ExternalInput")
    outwT_d = nc.dram_tensor("outwT", [128, 2, D], BF16, kind="ExternalInput")
    outb_d = nc.dram_tensor("outb", [D, 1], F32, kind="ExternalInput")
    yt_d = nc.dram_tensor("yt", [D, t_dec + 1, BL], F32, kind="ExternalOutput")

    with ExitStack() as ctx:
        tc = ctx.enter_context(tile.TileContext(nc))
        singles = ctx.enter_context(tc.tile_pool(name="singles", bufs=1))
        xpool = ctx.enter_context(tc.tile_pool(name="xpool", bufs=6))
        work = ctx.enter_context(tc.tile_pool(name="work", bufs=3))
        hpool = ctx.enter_context(tc.tile_pool(name="hpool", bufs=2))
        cpool = ctx.enter_context(tc.tile_pool(name="cpool", bufs=2))
        oap = ctx.enter_context(tc.tile_pool(name="oap", bufs=2))
        gpool = ctx.enter_context(
            tc.tile_pool(name="gpool", bufs=3, space=bass.MemorySpace.PSUM))
        gbc = ctx.enter_context(
            tc.tile_pool(name="gbc", bufs=2, space=bass.MemorySpace.PSUM))
        tpp = ctx.enter_context(
            tc.tile_pool(name="tpp", bufs=1, space=bass.MemorySpace.PSUM))

        # ---- persistent constants ----
        s_wxenc = singles.tile([66, 8, 128], BF16)
        nc.sync.dma_start(s_wxenc, wxenc_d[:, :, :])
        s_wxdec = singles.tile([66, 8, 128], BF16)
        nc.sync.dma_start(s_wxdec, wxdec_d[:, :, :])
        s_whhenc = singles.tile([128, 2, 8, 128], BF16)
        nc.sync.dma_start(s_whhenc, whhenc_d[:, :, :, :])
        s_whhdec = singles.tile([128, 2, 8, 128], BF16)
        nc.sync.dma_start(s_whhdec, whhdec_d[:, :, :, :])
        s_wcomb = singles.tile([128, 2, 8, 128], BF16)
        nc.sync.dma_start(s_wcomb, wcomb_d[:, :, :, :])
        s_biasA = singles.tile([4, 3, 128], BF16)
        nc.sync.dma_start(s_biasA, biasA_d[:, :, :])
        s_biasB = singles.tile([2, 3, 128], BF16)
        nc.sync.dma_start(s_biasB, biasB_d[:, :, :])
        s_biasC = singles.tile([2, 3, 128], BF16)
        nc.sync.dma_start(s_biasC, biasC_d[:, :, :])
        s_obT = singles.tile([1, D], BF16)
        nc.sync.dma_start(s_obT, obT_d[:, :])
        s_onesy = singles.tile([1, 512], BF16)
        nc.sync.dma_start(s_onesy, onesy_d[:, :])
        s_bonesA = singles.tile([4, 256], BF16)
        nc.sync.dma_start(s_bonesA, blockones_d[0:4, 0:256])
        s_bonesBC = singles.tile([2, 128], BF16)
        nc.sync.dma_start(s_bonesBC, blockones_d[4:6, 256:384])
        s_identb = singles.tile([128, 128], BF16)
        nc.sync.dma_start(s_identb, ident_d[:, :])
        s_edup = singles.tile([128, t_enc], F32)
        nc.sync.dma_start(s_edup, edup_d[:, :])
        s_outwT = singles.tile([128, 2, D], BF16)
        nc.sync.dma_start(s_outwT, outwT_d[:, :, :])
        s_outb = singles.tile([D, 1], f)
        nc.sync.dma_start(s_outb, outb_d[:, :])
        s_x0p = singles.tile([66, BL], BF16)
        nc.sync.dma_start(s_x0p, x0p_d[:, :])

        # ---- initial state ----
        c_prev = singles.tile([128, 2, BL], f, tag="c0")
        nc.vector.memset(c_prev, 0.0)
        hT_i0 = singles.tile([128, BL], BF16, tag="hi0")
        nc.vector.memset(hT_i0, 0.0)
        hT_i1 = singles.tile([128, BL], BF16, tag="hi1")
        nc.vector.memset(hT_i1, 0.0)
        hT_prev = (hT_i0, hT_i1)
        o_acc = singles.tile([128, 128], BF16, tag="oacc0")
        nc.vector.memset(o_acc, 0.0)

        def chain(insts):
            for a, b in zip(insts[1:], insts[:-1]):
                add_dep_helper(a.ins, b.ins, sync=False, reason="pe-order")

        # gate chunk m -> (bank, col offset): A=i,f (m0-3), B=g (m6,7), C=o (m4,5)
        def bank_slice(psA, psB, psC, m):
            if m < 4:
                return psA[:, 64 * m:64 * m + 64]
            if m >= 6:
                return psB[:, 64 * (m - 6):64 * (m - 6) + 64]
            return psC[:, 64 * (m - 4):64 * (m - 4) + 64]

        def gate_mms(psA, psB, psC, whh, bset, xlhs=None, xrhs=None):
            """All matmuls of one step. Gates split across three PSUM banks
            so each ACT read waits only on its own bank's writers (PSUM
            bank-level write/read serialization is a hardware constraint).
            One start=True (bias) matmul opens each bank; bias + x MMs run in
            the PE-idle window; h MMs go bank-A-first. The explicit chain
            pins the scheduler to this PE order."""
            mms = [
                nc.tensor.matmul(psA, s_biasA[:, bset, :], s_bonesA,
                                 start=True, stop=False, skip_group_check=True),
                nc.tensor.matmul(psB, s_biasB[:, bset, :], s_bonesBC,
                                 start=True, stop=False, skip_group_check=True),
                nc.tensor.matmul(psC, s_biasC[:, bset, :], s_bonesBC,
                                 start=True, stop=False, skip_group_check=True),
            ]
            if xlhs is not None:
                for m in (0, 1, 2, 3, 6, 7, 4, 5):
                    mms.append(nc.tensor.matmul(bank_slice(psA, psB, psC, m),
                                                xlhs[:, m, :], xrhs,
                                                start=False, stop=False,
                                                skip_group_check=True))
            # bank-major, k-interleaved: bank A's k1 half issues right as h1
            # lands (h1 is ready ~240ns after h0, before the k0-A subgroup
            # drains), so sig_if's bank completes ~180ns earlier than the
            # all-k0-then-all-k1 order
            for ms in ((0, 1, 2, 3), (6, 7), (4, 5)):
                for k in (0, 1):
                    for m in ms:
                        mms.append(nc.tensor.matmul(
                            bank_slice(psA, psB, psC, m),
                            whh[:, k, m, :], hT_prev[k],
                            start=False, stop=(k == 1),
                            skip_group_check=True))
            chain(mms)
            return mms[-1]

        def cell(psA, psB, psC, enc_t):
            """LSTM cell elementwise phase. Updates c_prev/hT_prev (+o_acc)."""
            nonlocal c_prev, hT_prev, o_acc
            if_t = work.tile([128, 256], BF16, tag="ift")
            nc.scalar.activation(if_t, psA, Sig)
            g_t = work.tile([128, 128], BF16, tag="gt")
            nc.scalar.activation(g_t, psB, Tanh)
            o_t = work.tile([128, 128], BF16, tag="ot")
            nc.scalar.activation(o_t, psC, Sig)
            c_new = cpool.tile([128, 2, BL], f, tag="c")
            tct = work.tile([128, 2, BL], BF16, tag="tct")
            hT_new = (hpool.tile([128, BL], BF16, tag="hT0", name="hT0"),
                      hpool.tile([128, BL], BF16, tag="hT1", name="hT1"))
            for k in (0, 1):
                sl = slice(64 * k, 64 * k + 64)
                fc = work.tile([128, BL], f, tag=f"fc{k}")
                nc.vector.tensor_mul(fc, if_t[:, 128 + 64 * k:192 + 64 * k],
                                     c_prev[:, k, :])
                ig = work.tile([128, BL], f, tag=f"ig{k}")
                nc.vector.tensor_mul(ig, if_t[:, sl], g_t[:, sl])
                nc.vector.tensor_add(c_new[:, k, :], fc, ig)
                nc.scalar.activation(tct[:, k, :], c_new[:, k, :], Tanh)
                nc.vector.tensor_mul(hT_new[k], o_t[:, sl], tct[:, k, :])
            if enc_t is not None:
                pending_o[0] = (o_t, enc_t)
            c_prev = c_new
            hT_prev = hT_new

        pending_o = [None]

        def flush_oacc(after=None):
            """Deferred o_acc capture: the PE transpose of step t's o is
            pinned after step t+1's matmuls so it never blocks the PE FIFO
            while waiting on sig_o."""
            nonlocal o_acc
            if pending_o[0] is None:
                return
            o_t, t = pending_o[0]
            pending_o[0] = None
            tp = tpp.tile([128, 128], BF16, tag="tp")
            tri = nc.tensor.transpose(tp, o_t, s_identb)
            if after is not None:
                add_dep_helper(tri.ins, after.ins, sync=False,
                               reason="defer transpose")
            o_acc2 = oap.tile([128, 128], BF16, tag="oacc")
            nc.vector.scalar_tensor_tensor(
                o_acc2, tp, s_edup[:, t:t + 1], o_acc, MUL, ADD)
            o_acc = o_acc2

        # ================= ENCODER =================
        for t in range(t_enc):
            xp_t = xpool.tile([66, BL], BF16, tag="xp")
            nc.sync.dma_start(xp_t, xp_d[t, :, :])
            psA = gpool.tile([128, 256], f, tag="gA")
            psB = gbc.tile([128, 128], f, tag="gB")
            psC = gbc.tile([128, 128], f, tag="gC")
            last_mm = gate_mms(psA, psB, psC, s_whhenc, 0,
                               xlhs=s_wxenc, xrhs=xp_t)
            flush_oacc(after=last_mm)
            if debug and t == 0:
                gcp = work.tile([128, 512], f, tag="gdbg")
                nc.vector.tensor_copy(gcp[:, 0:256], psA)
                nc.vector.tensor_copy(gcp[:, 384:512], psB)
                nc.vector.tensor_copy(gcp[:, 256:384], psC)
                nc.sync.dma_start(gdbg_d[:, :], gcp)
            cell(psA, psB, psC, t)

        if debug:
            nc.sync.dma_start(cdbg_d[:, :, :], c_prev)
            nc.sync.dma_start(hdbg_d[:, 0, :], hT_prev[0])
            nc.sync.dma_start(hdbg_d[:, 1, :], hT_prev[1])
            nc.sync.dma_start(odbg_d[:, :], o_acc)

        flush_oacc()

        # ===== boundary: hT_enc = o_sel^T * tanh(c_final) =====
        tce = work.tile([128, 2, BL], BF16, tag="tct")
        nc.scalar.activation(tce, c_prev, Tanh)
        tpe = tpp.tile([128, 128], BF16, tag="tp")
        nc.tensor.transpose(tpe, o_acc, s_identb)
        o_selT = work.tile([128, 128], BF16, tag="osel")
        nc.vector.tensor_copy(o_selT, tpe)
        hT_b = (hpool.tile([128, BL], BF16, tag="hT0", name="hTb0"),
                hpool.tile([128, BL], BF16, tag="hT1", name="hTb1"))
        for k in (0, 1):
            nc.vector.tensor_mul(hT_b[k], o_selT[:, 64 * k:64 * k + 64],
                                 tce[:, k, :])
        hT_prev = hT_b
        if debug:
            nc.sync.dma_start(hbdbg_d[:, 0, :], hT_b[0])
            nc.sync.dma_start(hbdbg_d[:, 1, :], hT_b[1])

        # ================= DECODER =================
        # y = out_W @ h + out_b computed in-loop: 2 small matmuls per step
        # accumulate into a persistent PSUM group bank (8 steps per bank,
        # opened by a rank-1 out_b matmul); one ACT copy + one DMA per group.
        psy_box = [None]
        pending_y = [None]

        def flush_y(after=None):
            """Deferred y matmuls: step j's out_W@h_j runs behind step j+1's
            gate matmuls in the PE FIFO so it never delays the next burst."""
            if pending_y[0] is None:
                return
            j, hT = pending_y[0]
            pending_y[0] = None
            g8 = j % 8
            ymms = [] if after is None else [after]
            if g8 == 0:
                psy_box[0] = tpp.tile([D, 512], f, tag="tp", name=f"psy{j}")
                ymms.append(nc.tensor.matmul(psy_box[0], s_obT, s_onesy,
                                             start=True, stop=False,
                                             skip_group_check=True))
            psy = psy_box[0]
            for k in (0, 1):
                ymms.append(nc.tensor.matmul(
                    psy[:, 64 * g8:64 * g8 + 64], s_outwT[:, k, :],
                    hT[k], start=False, stop=(k == 1),
                    skip_group_check=True))
            chain(ymms)
            if g8 == 7 or j == t_dec - 1:
                cnt = g8 + 1
                y_sb = work.tile([D, 512], f, tag="ysb")
                nc.scalar.copy(y_sb[:, 0:64 * cnt], psy[:, 0:64 * cnt])
                nc.sync.dma_start(yt_d[:, j - g8 + 1:j + 2, :],
                                  y_sb[:, 0:64 * cnt])

        for j in range(t_dec):
            psA = gpool.tile([128, 256], f, tag="gA")
            psB = gbc.tile([128, 128], f, tag="gB")
            psC = gbc.tile([128, 128], f, tag="gC")
            if j == 0:
                last_mm = gate_mms(psA, psB, psC, s_whhdec, 1,
                                   xlhs=s_wxdec, xrhs=s_x0p)
            else:
                last_mm = gate_mms(psA, psB, psC, s_wcomb, 2)
            flush_y(after=last_mm)
            cell(psA, psB, psC, None)
            pending_y[0] = (j, hT_prev)
        flush_y()

    nc.compile()
    return nc


def _prep_host(inputs, t_enc=T, t_dec=TDEC):
    """Build per-core in_maps from full inputs (numpy)."""
    x = np.asarray(inputs["input_tensor"], np.float32)
    tgt = np.asarray(inputs["target_tensor"], np.float32)
    lens = np.asarray(inputs["lens"]).astype(np.int64)

    eWih = np.asarray(inputs["enc_Wih"], np.float32)
    eWhh = np.asarray(inputs["enc_Whh"], np.float32)
    eb = (np.asarray(inputs["enc_bih"], np.float32)
          + np.asarray(inputs["enc_bhh"], np.float32))
    dWih = np.asarray(inputs["dec_Wih"], np.float32)
    dWhh = np.asarray(inputs["dec_Whh"], np.float32)
    db = (np.asarray(inputs["dec_bih"], np.float32)
          + np.asarray(inputs["dec_bhh"], np.float32))
    oW = np.asarray(inputs["out_W"], np.float32)
    ob = np.asarray(inputs["out_b"], np.float32)

    wcomb_full = dWhh + dWih @ oW          # [G4, H]
    bcomb = db + dWih @ ob                 # [G4]

    def chunked_x(W, freeze_big):
        # -> [66, 8, 128]: rows 0:64 x-weights^T, row 64 unused, row 65 freeze
        out = np.zeros((66, 8, 128), np.float32)
        for m, (r0, r1) in enumerate(CHUNK_ROWS):
            out[0:64, m, :] = W[r0:r1, :].T
            if freeze_big and m in (0, 1):
                out[65, m, :] = -BIG
            elif freeze_big and m in (2, 3):
                out[65, m, :] = BIG
        return out.astype(BF)

    def chunked_b(b):
        return np.stack([b[r0:r1] for (r0, r1) in CHUNK_ROWS])

    def chunked_h(W):
        # -> [128, 2, 8, 128]
        out = np.zeros((128, 2, 8, 128), np.float32)
        for m, (r0, r1) in enumerate(CHUNK_ROWS):
            for k in (0, 1):
                out[:, k, m, :] = W[r0:r1, 128 * k:128 * (k + 1)].T
        return out.astype(BF)

    wxenc = chunked_x(eWih, True)
    wxdec = chunked_x(dWih, False)
    whhenc = chunked_h(eWhh)
    whhdec = chunked_h(dWhh)
    wcomb = chunked_h(wcomb_full)
    # bias sets: 0=enc, 1=dec step0, 2=comb. Banks: A=m0-3, B=m6-7, C=m4-5.
    bsets = [chunked_b(eb), chunked_b(db), chunked_b(bcomb)]  # each [8, 128]
    biasA = np.stack([bs[0:4] for bs in bsets], 1).astype(BF)  # [4, 3, 128]
    biasB = np.stack([bs[6:8] for bs in bsets], 1).astype(BF)  # [2, 3, 128]
    biasC = np.stack([bs[4:6] for bs in bsets], 1).astype(BF)  # [2, 3, 128]
    obT = ob[None, :].astype(BF)
    onesy = np.ones((1, 512), np.float32).astype(BF)
    # blockones: rows 0-3 x cols 0-255 = 4x64 block-diag (bank A opener);
    # rows 4-5 x cols 256-383 = 2x64 block-diag (bank B/C opener)
    blockones = np.zeros((8, 512), np.float32)
    for m in range(4):
        blockones[m, 64 * m:64 * m + 64] = 1.0
    for m in range(2):
        blockones[4 + m, 256 + 64 * m:256 + 64 * m + 64] = 1.0
    blockones = blockones.astype(BF)
    ident = np.eye(128, dtype=np.float32).astype(BF)
    outwT = oW.T.reshape(2, 128, D).transpose(1, 0, 2).astype(BF).copy()
    outb = ob[:, None].astype(np.float32).copy()

    tt = np.arange(t_enc)[None, :]
    in_maps = []
    for c in range(NCORES):
        b0 = c * BL
        xs = x[b0:b0 + BL, :t_enc, :]                # [BL,t,D]
        xp = np.empty((t_enc, 66, BL), np.float32)
        xp[:, 0:D, :] = xs.transpose(1, 2, 0)
        xp[:, D, :] = 1.0
        lc = lens[b0:b0 + BL]
        mbar = (tt >= lc[:, None]).astype(np.float32)   # [BL,t]
        xp[:, D + 1, :] = mbar.T
        efreeze = (tt == (lc[:, None] - 1)).astype(np.float32)  # [BL,t]
        edup = np.concatenate([efreeze, efreeze], 0)    # [128,t]
        x0p = np.zeros((66, BL), np.float32)
        x0p[0:D, :] = tgt[b0:b0 + BL, 0, :].T
        x0p[D, :] = 1.0
        in_maps.append({
            "xp": np.ascontiguousarray(xp).astype(BF),
            "x0p": x0p.astype(BF),
            "wxenc": wxenc, "wxdec": wxdec,
            "whhenc": whhenc, "whhdec": whhdec, "wcomb": wcomb,
            "biasA": biasA, "biasB": biasB, "biasC": biasC,
            "obT": obT, "onesy": onesy,
            "blockones": blockones, "ident": ident,
            "edup": np.ascontiguousarray(edup),
            "outwT": outwT, "outb": outb,
        })
    return in_maps, lens


def kernel(**inputs) -> np.ndarray:
    global _PROGRAM, LAST_RESULTS
    if _PROGRAM is None:
        _PROGRAM = build_program()
    nc = _PROGRAM
    in_maps, lens = _prep_host(inputs)
    res = run_bass_kernel_spmd(nc, in_maps, core_ids=list(range(NCORES)))
    LAST_RESULTS = res
    out = np.zeros((B, T, D), np.float32)
    for c in range(NCORES):
        yt = res.results[c]["yt"]                      # [D, T, BL]
        out[c * BL:(c + 1) * BL] = yt.transpose(2, 1, 0)
    mask = (np.arange(T)[None, :] < lens[:, None])[:, :, None]
    out *= mask
    out[:, 0, :] = 0.0
    return out


# revision 48
# speedup vs baseline: 1.2798x; 1.0413x over previous
"""Trainium2 Bass kernel for the LSTM seq2seq autoencoder (layout B).

Strategy:
  - Data-parallel over batch: B=512 -> 64 rows per core on 8 cores.
  - Gates-on-partitions layout: gate preactivations live in one PSUM bank
    [128, 512] = 8 chunks x 64 batch cols, chunk order [i0 i1 f0 f1 o0 o1 g0 g1].
    Each step: 16 (h) + 8 (x, encoder) LDW+MM pairs of N=64 (FWL-eligible
    bf16 weights, LDWEIGHTS hidden behind matmuls) + one rank-8 bias matmul
    (decoder) whose rhs is a block-indicator [8, 512].
  - h^T [128, 2, 64] is produced directly by the DVE h-mul (no per-step
    transposes or PSUM->SBUF copies) and is the rhs of the next step's MMs.
  - Encoder length masking: c frozen by forcing i -> -BIG, f -> +BIG via the
    mbar row of xp; o captured at the freeze step via PE transpose +
    one fused scalar_tensor_tensor (o_acc = o^T_t * e_t + o_acc).
  - Decoder feedback folded: W_comb = Whh + Wih_dec @ out_W.
  - y = out_W @ h + out_b deferred entirely to a batched end-phase GEMM over
    h^T tiles dumped to DRAM each step (DMA engines are otherwise idle).
"""

import numpy as np
import ml_dtypes
from contextlib import ExitStack

import concourse.bass as bass
import concourse.bacc as bacc
import concourse.mybir as mybir
import concourse.tile as tile
from concourse.tile import add_dep_helper
from concourse.bass_utils import run_bass_kernel_spmd

B, T, D, H = 512, 512, 64, 256
G4 = 4 * H  # 1024
NCORES = 8
BL = B // NCORES  # 64
TDEC = T - 1      # 511 decoder steps
BIG = 30000.0
F32 = mybir.dt.float32
BF16 = mybir.dt.bfloat16
BF = ml_dtypes.bfloat16

_PROGRAM = None
LAST_RESULTS = None

# chunk order on the 512 free cols: [i0 i1 f0 f1 o0 o1 g0 g1]
# torch gate rows: i=[0,256) f=[256,512) g=[512,768) o=[768,1024)
CHUNK_ROWS = [(0, 128), (128, 256), (256, 384), (384, 512),
              (768, 896), (896, 1024), (512, 640), (640, 768)]

Sig = mybir.ActivationFunctionType.Sigmoid
Tanh = mybir.ActivationFunctionType.Tanh
Ident = mybir.ActivationFunctionType.Identity
MUL = mybir.AluOpType.mult
ADD = mybir.AluOpType.add


def build_program(t_enc=T, t_dec=TDEC, debug=False):
    nc = bacc.Bacc(None, target_bir_lowering=False)
    f = F32
    if debug:
        gdbg_d = nc.dram_tensor("gdbg", [128, 512], F32, kind="ExternalOutput")
        cdbg_d = nc.dram_tensor("cdbg", [128, 2, BL], F32, kind="ExternalOutput")
        hdbg_d = nc.dram_tensor("hdbg", [128, 2, BL], BF16, kind="ExternalOutput")
        odbg_d = nc.dram_tensor("odbg", [128, 128], BF16, kind="ExternalOutput")
        hbdbg_d = nc.dram_tensor("hbdbg", [128, 2, BL], BF16, kind="ExternalOutput")
    xp_d = nc.dram_tensor("xp", [t_enc, 66, BL], BF16, kind="ExternalInput")
    x0p_d = nc.dram_tensor("x0p", [66, BL], BF16, kind="ExternalInput")
    wxenc_d = nc.dram_tensor("wxenc", [66, 8, 128], BF16, kind="ExternalInput")
    wxdec_d = nc.dram_tensor("wxdec", [66, 8, 128], BF16, kind="ExternalInput")
    whhenc_d = nc.dram_tensor("whhenc", [128, 2, 8, 128], BF16, kind="ExternalInput")
    whhdec_d = nc.dram_tensor("whhdec", [128, 2, 8, 128], BF16, kind="ExternalInput")
    wcomb_d = nc.dram_tensor("wcomb", [128, 2, 8, 128], BF16, kind="ExternalInput")
    biasA_d = nc.dram_tensor("biasA", [4, 3, 128], BF16, kind="ExternalInput")
    biasB_d = nc.dram_tensor("biasB", [2, 3, 128], BF16, kind="ExternalInput")
    biasC_d = nc.dram_tensor("biasC", [2, 3, 128], BF16, kind="ExternalInput")
    obT_d = nc.dram_tensor("obT", [1, D], BF16, kind="ExternalInput")
    onesy_d = nc.dram_tensor("onesy", [1, 512], BF16, kind="ExternalInput")
    blockones_d = nc.dram_tensor("blockones", [8, 512], BF16, kind="ExternalInput")
    ident_d = nc.dram_tensor("ident", [128, 128], BF16, kind="ExternalInput")
    edup_d = nc.dram_tensor("edup", [128, t_enc], F32, kind="ExternalInput")
    outwT_d = nc.dram_tensor("outwT", [128, 2, D], BF16, kind="ExternalInput")
    outb_d = nc.dram_tensor("outb", [D, 1], F32, kind="ExternalInput")
    yt_d = nc.dram_tensor("yt", [D, t_dec + 1, BL], F32, kind="ExternalOutput")

    with ExitStack() as ctx:
        tc = ctx.enter_context(tile.TileContext(nc))
        singles = ctx.enter_context(tc.tile_pool(name="singles", bufs=1))
        xpool = ctx.enter_context(tc.tile_pool(name="xpool", bufs=6))
        work = ctx.enter_context(tc.tile_pool(name="work", bufs=3))
        hpool = ctx.enter_context(tc.tile_pool(name="hpool", bufs=2))
        cpool = ctx.enter_context(tc.tile_pool(name="cpool", bufs=2))
        oap = ctx.enter_context(tc.tile_pool(name="oap", bufs=2))
        gpool = ctx.enter_context(
            tc.tile_pool(name="gpool", bufs=3, space=bass.MemorySpace.PSUM))
        gbc = ctx.enter_context(
            tc.tile_pool(name="gbc", bufs=2, space=bass.MemorySpace.PSUM))
        tpp = ctx.enter_context(
            tc.tile_pool(name="tpp", bufs=1, space=bass.MemorySpace.PSUM))

        # ---- persistent constants ----
        s_wxenc = singles.tile([66, 8, 128], BF16)
        nc.sync.dma_start(s_wxenc, wxenc_d[:, :, :])
        s_wxdec = singles.tile([66, 8, 128], BF16)
        nc.sync.dma_start(s_wxdec, wxdec_d[:, :, :])
        s_whhenc = singles.tile([128, 2, 8, 128], BF16)
        nc.sync.dma_start(s_whhenc, whhenc_d[:, :, :, :])
        s_whhdec = singles.tile([128, 2, 8, 128], BF16)
        nc.sync.dma_start(s_whhdec, whhdec_d[:, :, :, :])
        s_wcomb = singles.tile([128, 2, 8, 128], BF16)
        nc.sync.dma_start(s_wcomb, wcomb_d[:, :, :, :])
        s_biasA = singles.tile([4, 3, 128], BF16)
        nc.sync.dma_start(s_biasA, biasA_d[:, :, :])
        s_biasB = singles.tile([2, 3, 128], BF16)
        nc.sync.dma_start(s_biasB, biasB_d[:, :, :])
        s_biasC = singles.tile([2, 3, 128], BF16)
        nc.sync.dma_start(s_biasC, biasC_d[:, :, :])
        s_obT = singles.tile([1, D], BF16)
        nc.sync.dma_start(s_obT, obT_d[:, :])
        s_onesy = singles.tile([1, 512], BF16)
        nc.sync.dma_start(s_onesy, onesy_d[:, :])
        s_bonesA = singles.tile([4, 256], BF16)
        nc.sync.dma_start(s_bonesA, blockones_d[0:4, 0:256])
        s_bonesBC = singles.tile([2, 128], BF16)
        nc.sync.dma_start(s_bonesBC, blockones_d[4:6, 256:384])
        s_identb = singles.tile([128, 128], BF16)
        nc.sync.dma_start(s_identb, ident_d[:, :])
        s_edup = singles.tile([128, t_enc], F32)
        nc.sync.dma_start(s_edup, edup_d[:, :])
        s_outwT = singles.tile([128, 2, D], BF16)
        nc.sync.dma_start(s_outwT, outwT_d[:, :, :])
        s_outb = singles.tile([D, 1], f)
        nc.sync.dma_start(s_outb, outb_d[:, :])
        s_x0p = singles.tile([66, BL], BF16)
        nc.sync.dma_start(s_x0p, x0p_d[:, :])

        # ---- initial state ----
        c_prev = singles.tile([128, 2, BL], f, tag="c0")
        nc.vector.memset(c_prev, 0.0)
        hT_i0 = singles.tile([128, BL], BF16, tag="hi0")
        nc.vector.memset(hT_i0, 0.0)
        hT_i1 = singles.tile([128, BL], BF16, tag="hi1")
        nc.vector.memset(hT_i1, 0.0)
        hT_prev = (hT_i0, hT_i1)
        o_acc = singles.tile([128, 128], BF16, tag="oacc0")
        nc.vector.memset(o_acc, 0.0)

        def chain(insts):
            for a, b in zip(insts[1:], insts[:-1]):
                add_dep_helper(a.ins, b.ins, sync=False, reason="pe-order")

        # gate chunk m -> (bank, col offset): A=i,f (m0-3), B=g (m6,7), C=o (m4,5)
        def bank_slice(psA, psB, psC, m):
            if m < 4:
                return psA[:, 64 * m:64 * m + 64]
            if m >= 6:
                return psB[:, 64 * (m - 6):64 * (m - 6) + 64]
            return psC[:, 64 * (m - 4):64 * (m - 4) + 64]

        def gate_mms(psA, psB, psC, whh, bset, xlhs=None, xrhs=None):
            """All matmuls of one step. Gates split across three PSUM banks
            so each ACT read waits only on its own bank's writers (PSUM
            bank-level write/read serialization is a hardware constraint).
            One start=True (bias) matmul opens each bank; bias + x MMs run in
            the PE-idle window; h MMs go bank-A-first. The explicit chain
            pins the scheduler to this PE order."""
            mms = [
                nc.tensor.matmul(psA, s_biasA[:, bset, :], s_bonesA,
                                 start=True, stop=False, skip_group_check=True),
                nc.tensor.matmul(psB, s_biasB[:, bset, :], s_bonesBC,
                                 start=True, stop=False, skip_group_check=True),
                nc.tensor.matmul(psC, s_biasC[:, bset, :], s_bonesBC,
                                 start=True, stop=False, skip_group_check=True),
            ]
            if xlhs is not None:
                for m in (0, 1, 2, 3, 6, 7, 4, 5):
                    mms.append(nc.tensor.matmul(bank_slice(psA, psB, psC, m),
                                                xlhs[:, m, :], xrhs,
                                                start=False, stop=False,
                                                skip_group_check=True))
            # bank-major, k-interleaved: bank A's k1 half issues right as h1
            # lands (h1 is ready ~240ns after h0, before the k0-A subgroup
            # drains), so sig_if's bank completes ~180ns earlier than the
            # all-k0-then-all-k1 order
            for ms in ((0, 1, 2, 3), (6, 7), (4, 5)):
                for k in (0, 1):
                    for m in ms:
                        mms.append(nc.tensor.matmul(
                            bank_slice(psA, psB, psC, m),
                            whh[:, k, m, :], hT_prev[k],
                            start=False, stop=(k == 1),
                            skip_group_check=True))
            chain(mms)
            return mms[-1]

        def cell(psA, psB, psC, enc_t):
            """LSTM cell elementwise phase. Updates c_prev/hT_prev (+o_acc)."""
            nonlocal c_prev, hT_prev, o_acc
            if_t = work.tile([128, 256], BF16, tag="ift")
            nc.scalar.activation(if_t, psA, Sig)
            g_t = work.tile([128, 128], BF16, tag="gt")
            nc.scalar.activation(g_t, psB, Tanh)
            o_t = work.tile([128, 128], BF16, tag="ot")
            nc.scalar.activation(o_t, psC, Sig)
            c_new = cpool.tile([128, 2, BL], f, tag="c")
            tct = work.tile([128, 2, BL], BF16, tag="tct")
            hT_new = (hpool.tile([128, BL], BF16, tag="hT0", name="hT0"),
                      hpool.tile([128, BL], BF16, tag="hT1", name="hT1"))
            for k in (0, 1):
                sl = slice(64 * k, 64 * k + 64)
                fc = work.tile([128, BL], f, tag=f"fc{k}")
                nc.vector.tensor_mul(fc, if_t[:, 128 + 64 * k:192 + 64 * k],
                                     c_prev[:, k, :])
                ig = work.tile([128, BL], f, tag=f"ig{k}")
                nc.vector.tensor_mul(ig, if_t[:, sl], g_t[:, sl])
                nc.vector.tensor_add(c_new[:, k, :], fc, ig)
                last_act = nc.scalar.activation(tct[:, k, :], c_new[:, k, :],
                                                Tanh)
                nc.vector.tensor_mul(hT_new[k], o_t[:, sl], tct[:, k, :])
            if enc_t is not None:
                pending_o[0] = (o_t, enc_t)
            c_prev = c_new
            hT_prev = hT_new
            return last_act

        pending_o = [None]

        Copy = mybir.ActivationFunctionType.Copy

        def flush_oacc(after=None, after_act=None):
            """Deferred o_acc capture, fully off the critical engines: PE
            transpose pinned after step t+1's matmuls; mask-multiply rides an
            ACT Copy (per-partition scale=e) pinned after the cell's ACTs;
            the serial accumulate runs on the otherwise-idle GPSIMD."""
            nonlocal o_acc
            if pending_o[0] is None:
                return
            o_t, t = pending_o[0]
            pending_o[0] = None
            tp = tpp.tile([128, 128], BF16, tag="tp")
            tri = nc.tensor.transpose(tp, o_t, s_identb)
            if after is not None:
                add_dep_helper(tri.ins, after.ins, sync=False,
                               reason="defer transpose")
            oam = work.tile([128, 128], BF16, tag="oam")
            cp = nc.scalar.activation(oam, tp, Copy,
                                      scale=s_edup[:, t:t + 1])
            if after_act is not None:
                add_dep_helper(cp.ins, after_act.ins, sync=False,
                               reason="defer oacc copy")
            o_acc2 = oap.tile([128, 128], BF16, tag="oacc")
            nc.gpsimd.tensor_add(o_acc2, o_acc, oam)
            o_acc = o_acc2

        # ================= ENCODER =================
        for t in range(t_enc):
            xp_t = xpool.tile([66, BL], BF16, tag="xp")
            nc.sync.dma_start(xp_t, xp_d[t, :, :])
            psA = gpool.tile([128, 256], f, tag="gA")
            psB = gbc.tile([128, 128], f, tag="gB")
            psC = gbc.tile([128, 128], f, tag="gC")
            last_mm = gate_mms(psA, psB, psC, s_whhenc, 0,
                               xlhs=s_wxenc, xrhs=xp_t)
            if debug and t == 0:
                gcp = work.tile([128, 512], f, tag="gdbg")
                nc.vector.tensor_copy(gcp[:, 0:256], psA)
                nc.vector.tensor_copy(gcp[:, 384:512], psB)
                nc.vector.tensor_copy(gcp[:, 256:384], psC)
                nc.sync.dma_start(gdbg_d[:, :], gcp)
            last_act = cell(psA, psB, psC, t)
            flush_oacc(after=last_mm, after_act=last_act)

        if debug:
            nc.sync.dma_start(cdbg_d[:, :, :], c_prev)
            nc.sync.dma_start(hdbg_d[:, 0, :], hT_prev[0])
            nc.sync.dma_start(hdbg_d[:, 1, :], hT_prev[1])
            nc.sync.dma_start(odbg_d[:, :], o_acc)

        flush_oacc()

        # ===== boundary: hT_enc = o_sel^T * tanh(c_final) =====
        tce = work.tile([128, 2, BL], BF16, tag="tct")
        nc.scalar.activation(tce, c_prev, Tanh)
        tpe = tpp.tile([128, 128], BF16, tag="tp")
        nc.tensor.transpose(tpe, o_acc, s_identb)
        o_selT = work.tile([128, 128], BF16, tag="osel")
        nc.vector.tensor_copy(o_selT, tpe)
        hT_b = (hpool.tile([128, BL], BF16, tag="hT0", name="hTb0"),
                hpool.tile([128, BL], BF16, tag="hT1", name="hTb1"))
        for k in (0, 1):
            nc.vector.tensor_mul(hT_b[k], o_selT[:, 64 * k:64 * k + 64],
                                 tce[:, k, :])
        hT_prev = hT_b
        if debug:
            nc.sync.dma_start(hbdbg_d[:, 0, :], hT_b[0])
            nc.sync.dma_start(hbdbg_d[:, 1, :], hT_b[1])

        # ================= DECODER =================
        # y = out_W @ h + out_b computed in-loop: 2 small matmuls per step
        # accumulate into a persistent PSUM group bank (8 steps per bank,
        # opened by a rank-1 out_b matmul); one ACT copy + one DMA per group.
        psy_box = [None]
        pending_y = [None]

        def flush_y(after=None):
            """Deferred y matmuls: step j's out_W@h_j runs behind step j+1's
            gate matmuls in the PE FIFO so it never delays the next burst."""
            if pending_y[0] is None:
                return
            j, hT = pending_y[0]
            pending_y[0] = None
            g8 = j % 8
            ymms = [] if after is None else [after]
            if g8 == 0:
                psy_box[0] = tpp.tile([D, 512], f, tag="tp", name=f"psy{j}")
                ymms.append(nc.tensor.matmul(psy_box[0], s_obT, s_onesy,
                                             start=True, stop=False,
                                             skip_group_check=True))
            psy = psy_box[0]
            for k in (0, 1):
                ymms.append(nc.tensor.matmul(
                    psy[:, 64 * g8:64 * g8 + 64], s_outwT[:, k, :],
                    hT[k], start=False, stop=(k == 1),
                    skip_group_check=True))
            chain(ymms)
            if g8 == 7 or j == t_dec - 1:
                cnt = g8 + 1
                y_sb = work.tile([D, 512], f, tag="ysb")
                nc.scalar.copy(y_sb[:, 0:64 * cnt], psy[:, 0:64 * cnt])
                nc.sync.dma_start(yt_d[:, j - g8 + 1:j + 2, :],
                                  y_sb[:, 0:64 * cnt])

        for j in range(t_dec):
            psA = gpool.tile([128, 256], f, tag="gA")
            psB = gbc.tile([128, 128], f, tag="gB")
            psC = gbc.tile([128, 128], f, tag="gC")
            if j == 0:
                last_mm = gate_mms(psA, psB, psC, s_whhdec, 1,
                                   xlhs=s_wxdec, xrhs=s_x0p)
            else:
                last_mm = gate_mms(psA, psB, psC, s_wcomb, 2)
            flush_y(after=last_mm)
            cell(psA, psB, psC, None)
            pending_y[0] = (j, hT_prev)
        flush_y()

    nc.compile()
    return nc


def _prep_host(inputs, t_enc=T, t_dec=TDEC):
    """Build per-core in_maps from full inputs (numpy)."""
    x = np.asarray(inputs["input_tensor"], np.float32)
    tgt = np.asarray(inputs["target_tensor"], np.float32)
    lens = np.asarray(inputs["lens"]).astype(np.int64)

    eWih = np.asarray(inputs["enc_Wih"], np.float32)
    eWhh = np.asarray(inputs["enc_Whh"], np.float32)
    eb = (np.asarray(inputs["enc_bih"], np.float32)
          + np.asarray(inputs["enc_bhh"], np.float32))
    dWih = np.asarray(inputs["dec_Wih"], np.float32)
    dWhh = np.asarray(inputs["dec_Whh"], np.float32)
    db = (np.asarray(inputs["dec_bih"], np.float32)
          + np.asarray(inputs["dec_bhh"], np.float32))
    oW = np.asarray(inputs["out_W"], np.float32)
    ob = np.asarray(inputs["out_b"], np.float32)

    wcomb_full = dWhh + dWih @ oW          # [G4, H]
    bcomb = db + dWih @ ob                 # [G4]

    def chunked_x(W, freeze_big):
        # -> [66, 8, 128]: rows 0:64 x-weights^T, row 64 unused, row 65 freeze
        out = np.zeros((66, 8, 128), np.float32)
        for m, (r0, r1) in enumerate(CHUNK_ROWS):
            out[0:64, m, :] = W[r0:r1, :].T
            if freeze_big and m in (0, 1):
                out[65, m, :] = -BIG
            elif freeze_big and m in (2, 3):
                out[65, m, :] = BIG
        return out.astype(BF)

    def chunked_b(b):
        return np.stack([b[r0:r1] for (r0, r1) in CHUNK_ROWS])

    def chunked_h(W):
        # -> [128, 2, 8, 128]
        out = np.zeros((128, 2, 8, 128), np.float32)
        for m, (r0, r1) in enumerate(CHUNK_ROWS):
            for k in (0, 1):
                out[:, k, m, :] = W[r0:r1, 128 * k:128 * (k + 1)].T
        return out.astype(BF)

    wxenc = chunked_x(eWih, True)
    wxdec = chunked_x(dWih, False)
    whhenc = chunked_h(eWhh)
    whhdec = chunked_h(dWhh)
    wcomb = chunked_h(wcomb_full)
    # bias sets: 0=enc, 1=dec step0, 2=comb. Banks: A=m0-3, B=m6-7, C=m4-5.
    bsets = [chunked_b(eb), chunked_b(db), chunked_b(bcomb)]  # each [8, 128]
    biasA = np.stack([bs[0:4] for bs in bsets], 1).astype(BF)  # [4, 3, 128]
    biasB = np.stack([bs[6:8] for bs in bsets], 1).astype(BF)  # [2, 3, 128]
    biasC = np.stack([bs[4:6] for bs in bsets], 1).astype(BF)  # [2, 3, 128]
    obT = ob[None, :].astype(BF)
    onesy = np.ones((1, 512), np.float32).astype(BF)
    # blockones: rows 0-3 x cols 0-255 = 4x64 block-diag (bank A opener);
    # rows 4-5 x cols 256-383 = 2x64 block-diag (bank B/C opener)
    blockones = np.zeros((8, 512), np.float32)
    for m in range(4):
        blockones[m, 64 * m:64 * m + 64] = 1.0
    for m in range(2):
        blockones[4 + m, 256 + 64 * m:256 + 64 * m + 64] = 1.0
    blockones = blockones.astype(BF)
    ident = np.eye(128, dtype=np.float32).astype(BF)
    outwT = oW.T.reshape(2, 128, D).transpose(1, 0, 2).astype(BF).copy()
    outb = ob[:, None].astype(np.float32).copy()

    tt = np.arange(t_enc)[None, :]
    in_maps = []
    for c in range(NCORES):
        b0 = c * BL
        xs = x[b0:b0 + BL, :t_enc, :]                # [BL,t,D]
        xp = np.empty((t_enc, 66, BL), np.float32)
        xp[:, 0:D, :] = xs.transpose(1, 2, 0)
        xp[:, D, :] = 1.0
        lc = lens[b0:b0 + BL]
        mbar = (tt >= lc[:, None]).astype(np.float32)   # [BL,t]
        xp[:, D + 1, :] = mbar.T
        efreeze = (tt == (lc[:, None] - 1)).astype(np.float32)  # [BL,t]
        edup = np.concatenate([efreeze, efreeze], 0)    # [128,t]
        x0p = np.zeros((66, BL), np.float32)
        x0p[0:D, :] = tgt[b0:b0 + BL, 0, :].T
        x0p[D, :] = 1.0
        in_maps.append({
            "xp": np.ascontiguousarray(xp).astype(BF),
            "x0p": x0p.astype(BF),
            "wxenc": wxenc, "wxdec": wxdec,
            "whhenc": whhenc, "whhdec": whhdec, "wcomb": wcomb,
            "biasA": biasA, "biasB": biasB, "biasC": biasC,
            "obT": obT, "onesy": onesy,
            "blockones": blockones, "ident": ident,
            "edup": np.ascontiguousarray(edup),
            "outwT": outwT, "outb": outb,
        })
    return in_maps, lens


def kernel(**inputs) -> np.ndarray:
    global _PROGRAM, LAST_RESULTS
    if _PROGRAM is None:
        _PROGRAM = build_program()
    nc = _PROGRAM
    in_maps, lens = _prep_host(inputs)
    res = run_bass_kernel_spmd(nc, in_maps, core_ids=list(range(NCORES)))
    LAST_RESULTS = res
    out = np.zeros((B, T, D), np.float32)
    for c in range(NCORES):
        yt = res.results[c]["yt"]                      # [D, T, BL]
        out[c * BL:(c + 1) * BL] = yt.transpose(2, 1, 0)
    mask = (np.arange(T)[None, :] < lens[:, None])[:, :, None]
    out *= mask
    out[:, 0, :] = 0.0
    return out
